# revision 10
# baseline (speedup 1.0000x reference)
"""Trainium2 Bass kernel for multi-head attention (B=4, N=2048, C=1024, H=16).

Sharding: 8 cores = (batch b, query-half qh). Each core computes attention for
its 1024 query tokens of batch b against all 2048 keys of batch b, all 16
heads, plus the output projection. Host-side work is layout only (transpose /
concat); all FLOPs run on device.

Per-core layout: activations are feature-major ("xT" = [C, tokens]) so every
matmul contracts over the partition axis. Scores are computed transposed
(ST[j keys, i queries]) which makes softmax denominators cheap and makes P@V
need no transpose of P. Softmax skips max-subtraction (|S| <~ 25 for this
distribution, exp is safe in fp32). All matmul operands are bf16.

PE array tiling: the score matmuls contract over D=64, so the two heads of a
pair run as concurrent 64x128 row tiles (T0/T8, auto-derived from the base
partitions). The AV matmuls have 64 stationary columns per head, so the two
heads run as concurrent 128x64 column tiles (T0/T1) accumulating into one
[128,512] PSUM bank - head A dims on partitions 0:64, head B on 64:128,
which is exactly the O^T layout the output projection wants. The softmax
denominator (which previously rode along as a 65th ones-column of V, blocking
column tiling) is instead accumulated per-jt on the DVE (bf16 adds at 2x
rate) and collapsed across partitions by a single ones-stationary matmul per
(pair, query-block).

Matmuls that share a stationary operand are emitted back-to-back and a
post-schedule pass drops the redundant Ldweights (walrus emits one per
matmul unconditionally).
"""

import sys

import ml_dtypes
import numpy as np

BF = ml_dtypes.bfloat16

sys.path.insert(0, "/opt/trn_rl_repo")

import concourse.bass as bass  # noqa: E402
import concourse.bacc as bacc  # noqa: E402
import concourse.mybir as mybir  # noqa: E402
import concourse.tile as tile  # noqa: E402

# Problem constants (hardcoded by contract).
B, N, C = 4, 2048, 1024
H, D = 16, 64
SCALE = D**-0.5  # 0.125
T = N  # key tokens per core
TQ = N // 2  # query tokens per core
KC = C // 128  # contraction chunks of 128
PAIRS = H // 2  # head pairs; pair g = heads (2g, 2g+1)
NJT = T // 128  # 16 key tiles
F32 = mybir.dt.float32
BF16 = mybir.dt.bfloat16
EXP = mybir.ActivationFunctionType.Exp

_KEEP_PE = ("Matmult", "EventSemaphore", "Ldweights")


def _dedupe_ldweights(nc):
    """Drop Ldweights that reload the exact weights already resident.

    Runs after tile scheduling (program order fixed, waits materialized) and
    before finalize. An Ldweights is redundant iff it has the same source AP /
    tile position as the previous kept one, carries no waits or updates of its
    own, and its dependencies are a subset of the kept load's. Only Matmult
    (non-self-loading for bf16) and EventSemaphore may sit in between; any
    other PE instruction conservatively resets the tracked state.
    """
    removed = 0
    for b in nc.main_func.blocks:
        newlist = []
        last = None
        for i in b.instructions:
            if i.opcode == "Ldweights":
                ap = i.ins[0]
                sig = (
                    ap.memsetref,
                    ap.memref,
                    ap.offset,
                    str(ap.ap),
                    str(ap.dtype),
                    str(i.tile_position),
                    str(i.perf_mode),
                    str(getattr(i, "is_transpose", None)),
                )
                deps = {n for n, _ in i.sync_dependencies()}
                deps |= {n for n, _ in i.nosync_dependencies()}
                has_wait = bool(i.sync_info and i.sync_info.on_wait)
                has_upd = bool(i.sync_info and i.sync_info.on_update)
                if (
                    last is not None
                    and sig == last[0]
                    and not has_wait
                    and not has_upd
                    and deps <= last[1]
                ):
                    removed += 1
                    continue
                last = (sig, deps)
            elif i.engine == mybir.EngineType.PE and i.opcode not in _KEEP_PE:
                last = None
            newlist.append(i)
        b.instructions[:] = newlist
    return removed


def build_bass(reps=1, loop_iters=0, ablate=()):
    from contextlib import ExitStack

    nc = bacc.Bacc()
    xT = nc.dram_tensor("xT", [C, T], BF16, kind="ExternalInput")
    wkT = nc.dram_tensor("wkT", [PAIRS, 128, KC, 128], BF16, kind="ExternalInput")
    wqT = nc.dram_tensor("wqT", [PAIRS, 128, KC, 128], BF16, kind="ExternalInput")
    wvT = nc.dram_tensor("wvT", [128, KC, 4, 256], BF16, kind="ExternalInput")
    woT = nc.dram_tensor("woT", [128, KC, C], BF16, kind="ExternalInput")
    bo = nc.dram_tensor("bo", [C], F32, kind="ExternalInput")
    # ones_in[:, 0] = 1 and ones_in[:, 65] = 1, zeros elsewhere: slices
    # [:, 0:64] / [:, 64:128] are the denominator matmul stationaries that
    # place head A's / head B's key-sum on output partitions 0 / 1.
    ones_in = nc.dram_tensor("ones_in", [128, 128], BF16, kind="ExternalInput")
    outT = nc.dram_tensor("outT", [C, TQ], F32, kind="ExternalOutput")

    xT_r = xT.rearrange("(kc p) t -> p kc t", p=128)
    bo_r = bo.rearrange("(a p) -> p a", p=128)
    outT_r = outT.rearrange("(et p) i -> et p i", p=128)

    with tile.TileContext(nc) as tc, ExitStack() as ctx:
        ctx.enter_context(
            nc.allow_low_precision(reason="bf16 matmul path is within error budget")
        )
        const = ctx.enter_context(tc.tile_pool(name="const", bufs=1))
        wpool = ctx.enter_context(tc.tile_pool(name="wpool", bufs=2))
        wvp = ctx.enter_context(tc.tile_pool(name="wvp", bufs=1))
        kqp = ctx.enter_context(tc.tile_pool(name="kqp", bufs=2))
        vpool = ctx.enter_context(tc.tile_pool(name="vpool", bufs=1))
        # pt tiles live for a full block (16 jt) plus the production margin of
        # the next block: the consume side (AV + denominator adds) lags one
        # whole (pair, query-block) behind the produce side (ST + exp) so no
        # consumer ever waits on a just-produced tile.
        ptp = ctx.enter_context(tc.tile_pool(name="ptp", bufs=NJT + 4))
        accp = ctx.enter_context(tc.tile_pool(name="accp", bufs=2))
        smp = ctx.enter_context(tc.tile_pool(name="smp", bufs=2))
        obp = ctx.enter_context(tc.tile_pool(name="obp", bufs=2))
        # PSUM budget (8 banks): st2 2x[128,2,512] = 4, av 1, dn 1, proj 2.
        pp = ctx.enter_context(tc.tile_pool(name="pp", bufs=2, space="PSUM"))
        psA = ctx.enter_context(tc.tile_pool(name="psA", bufs=1, space="PSUM"))
        ps2 = ctx.enter_context(tc.tile_pool(name="ps2", bufs=2, space="PSUM"))
        drp = ctx.enter_context(tc.tile_pool(name="drp", bufs=2, space="DRAM"))

        import contextlib
        loop_ctx = (
            tc.For_i(0, loop_iters, 1) if loop_iters else contextlib.nullcontext()
        )
        with loop_ctx:
          for _rep in range(reps):
            # Constants first (small), then xT per c-chunk so the first
            # projection matmuls start as soon as chunk 0 lands.
            ones = const.tile([128, 128], BF16, tag="ones")
            nc.sync.dma_start(out=ones, in_=ones_in[:, :])
            bo_t = const.tile([128, KC], F32, tag="bo")
            nc.sync.dma_start(out=bo_t, in_=bo_r)
            xt = const.tile([128, KC, T], BF16, tag="xw")
            # O^T, concatenated over heads: rows fc*128+p = feature f, cols = query i.
            ot_t = const.tile([128, PAIRS, TQ], BF16, tag="ot")

            def kq_alloc(g):
                """Allocate tiles and start weight DMAs for pair g's k/q projections."""
                t = {}
                t["wk"] = wpool.tile([128, KC, 128], BF16, tag="wk", name=f"wk{g}")
                nc.sync.dma_start(out=t["wk"], in_=wkT[g])
                t["wq"] = wpool.tile([128, KC, 128], BF16, tag="wq", name=f"wq{g}")
                nc.sync.dma_start(out=t["wq"], in_=wqT[g])
                t["kt"] = kqp.tile([128, T], BF16, tag="kT", name=f"kt{g}")
                t["qt"] = kqp.tile([128, TQ], BF16, tag="qT", name=f"qt{g}")
                return t

            def v_emit_all(wv_t, v_t):
                """V projection for all 4 head-groups: per (tt, kc) one xt
                stationary serves 4 back-to-back matmuls (dedupe drops 3
                Ldweights). PSUM evacuation runs on the (otherwise idle)
                scalar engine."""
                if "proj" in ablate:
                    nc.vector.tensor_copy(v_t[:, 0, 0:4, :], xt[:, 0, 0:256])
                    yield
                    return
                for tt in range(NJT):
                    pvs = [
                        pp.tile([128, 256], F32, tag="ps", name=f"pv{tt}_0"),
                        pp.tile([128, 256], F32, tag="ps", name=f"pv{tt}_1"),
                        psA.tile([128, 256], F32, tag="av", name=f"pv{tt}_2"),
                        psA.tile([128, 256], F32, tag="dn", name=f"pv{tt}_3"),
                    ]
                    for kc in range(KC):
                        for p in range(4):
                            nc.tensor.matmul(
                                pvs[p],
                                xt[:, kc, tt * 128 : (tt + 1) * 128],
                                wv_t[:, kc, p, :],
                                start=(kc == 0),
                                stop=(kc == KC - 1),
                            )
                        yield
                    for p in range(4):
                        nc.scalar.copy(v_t[:, tt, 4 * p : 4 * p + 4, :], pvs[p])
                    yield

            def kq_emit(g, t):
                if "proj" in ablate:
                    nc.vector.tensor_copy(t["kt"][:, 0:512], xt[:, 0, 0:512])
                    nc.vector.tensor_copy(t["qt"][:, 0:512], xt[:, 1, 0:512])
                    yield
                    return
                kt_t = t["kt"]
                # Token-pair groups: one wk stationary serves 2 adjacent
                # matmuls (dedupe drops every other Ldweights).
                for tcn2 in range(T // 1024):
                    pks = [
                        pp.tile([128, 512], F32, tag="ps", name=f"pk{g}_{tcn2}_{h}")
                        for h in range(2)
                    ]
                    for kc in range(KC):
                        for h in range(2):
                            tcn = tcn2 * 2 + h
                            nc.tensor.matmul(
                                pks[h],
                                t["wk"][:, kc, :],
                                xt[:, kc, tcn * 512 : (tcn + 1) * 512],
                                start=(kc == 0),
                                stop=(kc == KC - 1),
                            )
                        yield
                    for h in range(2):
                        tcn = tcn2 * 2 + h
                        nc.vector.tensor_copy(
                            kt_t[:, tcn * 512 : (tcn + 1) * 512], pks[h]
                        )
                        yield
                qt_t = t["qt"]
                pqs = [
                    pp.tile([128, 512], F32, tag="ps", name=f"pq{g}_{icn}")
                    for icn in range(TQ // 512)
                ]
                for kc in range(KC):
                    for icn in range(TQ // 512):
                        nc.tensor.matmul(
                            pqs[icn],
                            t["wq"][:, kc, :],
                            xt[:, kc, icn * 512 : (icn + 1) * 512],
                            start=(kc == 0),
                            stop=(kc == KC - 1),
                        )
                    yield
                for icn in range(TQ // 512):
                    nc.vector.tensor_copy(
                        qt_t[:, icn * 512 : (icn + 1) * 512], pqs[icn]
                    )
                    yield

            # V projection for all groups runs up front. Pair 0's k/q weight
            # DMAs start first so they land before the V phase ends.
            tiles = [None] * PAIRS
            tiles[0] = kq_alloc(0)
            wv_t = wvp.tile([128, KC, 4, 256], BF16, tag="wv")
            nc.sync.dma_start(out=wv_t, in_=wvT[:, :, :, :])
            # V per head, 64-wide (no ones column): [p, jt, head, d].
            v_all = vpool.tile([128, NJT, H, 64], BF16, tag="v")
            for kc in range(KC):
                nc.sync.dma_start(out=xt[:, kc, :], in_=xT_r[:, kc, :])
            for _ in v_emit_all(wv_t, v_all):
                pass
            for _ in kq_emit(0, tiles[0]):
                pass

            def emit_tail(g, isl, av, acc):
                """Denominator + evacuation for one finished (pair, query-block).

                Emitted at the end of the block AFTER the one it belongs to
                (the consume side lags a full block), so acc and av are long
                done. Returns a closure for the final normalize mul, to be
                invoked a couple of steps later still - by then the
                reciprocal's DRAM-broadcast bounce has landed, so the DVE
                never blocks on it.
                """
                if "noepi" in ablate:
                    av_sb = smp.tile([128, 512], BF16, tag="avs", name=f"as{g}_{isl.start}")
                    nc.vector.tensor_copy(av_sb, av)
                    nc.vector.tensor_copy(ot_t[:, g, isl], av_sb)
                    return lambda: None
                # Collapse acc across the 128 key partitions with
                # ones-stationary matmuls; head A lands on output partition 0,
                # head B on partition 1 (128x64 mode, same as AV).
                dn = psA.tile([64, 512], F32, tag="dn", name=f"dn{g}_{isl.start}")
                nc.tensor.matmul(dn, ones[:, 0:64], acc[:, 0, :],
                                 start=True, stop=False)
                nc.tensor.matmul(dn, ones[:, 64:128], acc[:, 1, :],
                                 start=False, stop=True)
                # Copy accumulator to SBUF right away to free the av bank.
                av_sb = smp.tile([128, 512], BF16, tag="avs", name=f"as{g}_{isl.start}")
                nc.vector.tensor_copy(av_sb, av)
                rec = smp.tile([2, 512], BF16, tag="rec", name=f"rc{g}_{isl.start}")
                nc.vector.reciprocal(rec, dn[0:2, :])
                # Bounce through DRAM and broadcast-load across partitions
                # (stride-0 DRAM AP): rows 0:64 get 1/dA, 64:128 get 1/dB.
                rec_d = drp.tile([2, 512], BF16, tag="rd", name=f"rd{g}_{isl.start}")
                nc.sync.dma_start(out=rec_d, in_=rec)
                bc = smp.tile([128, 512], BF16, tag="bc", name=f"bc{g}_{isl.start}")
                nc.sync.dma_start(
                    out=bc[0:64, :],
                    in_=bass.AP(tensor=rec_d[:, :].tensor, offset=rec_d[0:1, :].offset,
                                ap=[[0, 64], [1, 512]]),
                )
                nc.sync.dma_start(
                    out=bc[64:128, :],
                    in_=bass.AP(tensor=rec_d[:, :].tensor, offset=rec_d[1:2, :].offset,
                                ap=[[0, 64], [1, 512]]),
                )

                def mul():
                    # Normalize both heads straight into O^T layout.
                    nc.vector.tensor_mul(ot_t[:, g, isl], av_sb, bc)

                return mul

            def consume_step(blk, av, acc, jt):
                """One lagged consume step: AV matmuls + denominator add for
                key-tile jt of the previous (pair, query-block)."""
                pg = blk["g"]
                pv = blk["pts"][jt]
                if "noav" not in ablate:
                    nc.tensor.matmul(
                        av[0:64, :], v_all[:, jt, 2 * pg, :], pv[:, 0, :],
                        start=(jt == 0), stop=(jt == NJT - 1),
                        skip_group_check=True,
                    )
                    nc.tensor.matmul(
                        av[64:128, :], v_all[:, jt, 2 * pg + 1, :], pv[:, 1, :],
                        start=(jt == 0), stop=(jt == NJT - 1),
                        skip_group_check=True,
                    )
                elif jt == 0:
                    nc.vector.tensor_copy(av[0:2, 0:2], pv[0:2, 0, 0:2])
                if "nodve" in ablate:
                    if jt == 0:
                        nc.vector.tensor_copy(acc[:, 0, 0:2], pv[:, 0, 0:2])
                elif jt == 0:
                    nc.vector.tensor_copy(acc, pv)
                else:
                    nc.vector.tensor_add(acc, acc, pv)

            # The attention pipeline: the produce side of block k (scores +
            # exp) runs while the consume side (AV accumulation, denominator
            # adds, epilogue) works on block k-1. Every cross-engine
            # dependency is therefore ~16 steps stale and no engine queue
            # ever blocks another.
            prev_blk = None
            pending_mul = None
            pending_tail = None
            for g in range(PAIRS):
                t = tiles[g]
                kt_t, qt_t = t["kt"], t["qt"]
                if g + 1 < PAIRS:
                    tiles[g + 1] = kq_alloc(g + 1)
                    kq_gen = kq_emit(g + 1, tiles[g + 1])
                else:
                    kq_gen = iter(())

                if "att" in ablate:
                    for _ in kq_gen:
                        pass
                    nc.vector.tensor_copy(ot_t[0:64, g, 0:512], kt_t[0:64, 0:512])
                    continue
                for icn in range(TQ // 512):
                    isl = slice(icn * 512, (icn + 1) * 512)
                    if prev_blk is not None:
                        # O^T accumulator for the PREVIOUS block, both heads
                        # via column tiling: head A dims on partitions 0:64
                        # (tile T0), head B on 64:128 (tile T1). One bank.
                        av = psA.tile([128, 512], F32, tag="av", name=f"av{g}_{icn}")
                        acc = accp.tile([128, 2, 512], BF16, tag="acc", name=f"ac{g}_{icn}")
                    pts = []
                    # 2-jt macro steps: group same-tile-mode matmuls (4 score
                    # MMs, then projections, then 4 AV MMs) so the PE array
                    # re-tiles 3 times per TWO key-tiles instead of per one.
                    for jm in range(NJT // 2):
                        sts = []
                        for h in range(2):
                            jt = 2 * jm + h
                            jsl = slice(jt * 128, (jt + 1) * 128)
                            # S^T[j, i] for both heads into one 2-bank PSUM
                            # tile; heads run as concurrent 64x128 row tiles.
                            st2 = ps2.tile([128, 2, 512], F32, tag="st2", name=f"st{g}_{icn}_{jt}")
                            nc.tensor.matmul(st2[:, 0, :], kt_t[0:64, jsl], qt_t[0:64, isl])
                            nc.tensor.matmul(st2[:, 1, :], kt_t[64:128, jsl], qt_t[64:128, isl])
                            sts.append(st2)
                        for h in range(2):
                            jt = 2 * jm + h
                            # One exp instruction covers both heads (1024 free).
                            pt2 = ptp.tile([128, 2, 512], BF16, tag="pt", name=f"pt{g}_{icn}_{jt}")
                            if "noact" in ablate:
                                nc.scalar.copy(pt2[:, 0, 0:2], sts[h][:, 0, 0:2])
                            else:
                                nc.scalar.activation(pt2[:, :, :], sts[h][:, :, :], EXP, scale=SCALE)
                            pts.append(pt2)
                        # Interleave next pair's projections while ACT runs
                        # (more during the fill block, which has no consume
                        # work).
                        for _ in range(4 if prev_blk is not None else 8):
                            next(kq_gen, None)
                        if prev_blk is not None:
                            consume_step(prev_blk, av, acc, 2 * jm)
                            consume_step(prev_blk, av, acc, 2 * jm + 1)
                        if jm == 1 and pending_mul is not None:
                            pending_mul()
                            pending_mul = None
                    if prev_blk is not None:
                        pending_mul = emit_tail(prev_blk["g"], prev_blk["isl"], av, acc)
                    prev_blk = {"g": g, "isl": isl, "pts": pts}
                for _ in kq_gen:
                    pass
            if "att" not in ablate:
                # Drain: consume the final block (no produce side left).
                av = psA.tile([128, 512], F32, tag="av", name="av_drain")
                acc = accp.tile([128, 2, 512], BF16, tag="acc", name="ac_drain")
                for jm in range(NJT // 2):
                    consume_step(prev_blk, av, acc, 2 * jm)
                    consume_step(prev_blk, av, acc, 2 * jm + 1)
                    if jm == 1 and pending_mul is not None:
                        pending_mul()
                        pending_mul = None
                emit_tail(prev_blk["g"], prev_blk["isl"], av, acc)()

            if "out" in ablate:
                ob0 = obp.tile([128, 512], F32, tag="ob", name="ob0")
                nc.vector.tensor_copy(ob0, ot_t[:, 0, 0:512])
                nc.sync.dma_start(out=outT_r[0, :, 0:512], in_=ob0)
                continue
            # Output projection: outT[e, i] = Wo @ O^T + bo. Both query halves
            # share each wo stationary (adjacent matmuls, dedupe).
            wo_t = const.tile([128, KC, C], BF16, tag="xw")
            nc.sync.dma_start(out=wo_t, in_=woT[:, :, :])
            for et in range(C // 128):
                pos = [
                    pp.tile([128, 512], F32, tag="ps", name=f"po{et}_{icn}")
                    for icn in range(TQ // 512)
                ]
                for fc in range(KC):
                    for icn in range(TQ // 512):
                        nc.tensor.matmul(
                            pos[icn],
                            wo_t[:, fc, et * 128 : (et + 1) * 128],
                            ot_t[:, fc, icn * 512 : (icn + 1) * 512],
                            start=(fc == 0),
                            stop=(fc == KC - 1),
                        )
                for icn in range(TQ // 512):
                    ob = obp.tile([128, 512], F32, tag="ob", name=f"o{et}_{icn}")
                    nc.vector.tensor_scalar_add(ob, pos[icn], bo_t[:, et : et + 1])
                    nc.sync.dma_start(
                        out=outT_r[et, :, icn * 512 : (icn + 1) * 512], in_=ob
                    )

    _dedupe_ldweights(nc)
    nc.finalize()
    return nc


_CACHE = {}


def _get_nc():
    if "nc" not in _CACHE:
        _CACHE["nc"] = build_bass()
    return _CACHE["nc"]


def make_in_maps(x, Wq, Wk, Wv, Wo, bo):
    """Host-side sharding: layout prep only (transposes / concatenation)."""
    x = np.asarray(x, dtype=np.float32)
    # Weights pre-tiled into the exact SBUF layouts (contiguous DMAs).
    # wk/wq: [g, p, kc, o] = W[g*128+o, kc*128+p]
    wkT = np.ascontiguousarray(
        np.asarray(Wk, np.float32).reshape(PAIRS, 128, KC, 128).transpose(0, 3, 2, 1)
    ).astype(BF)
    wqT = np.ascontiguousarray(
        np.asarray(Wq, np.float32).reshape(PAIRS, 128, KC, 128).transpose(0, 3, 2, 1)
    ).astype(BF)
    # wv: [p, kc, grp, col] where grp covers 4 heads x 64 dims = 256 cols
    wvT = np.ascontiguousarray(
        np.asarray(Wv, np.float32).T.reshape(KC, 128, 4, 256).transpose(1, 0, 2, 3)
    ).astype(BF)
    # wo: [p, fc, e] = Wo[e, fc*128+p]
    woT = np.ascontiguousarray(
        np.asarray(Wo, np.float32).T.reshape(KC, 128, C).transpose(1, 0, 2)
    ).astype(BF)
    bo = np.ascontiguousarray(np.asarray(bo, np.float32))
    ones_np = np.zeros((128, 128), BF)
    ones_np[:, 0] = 1  # denominator stationary A: head A sum -> partition 0
    ones_np[:, 65] = 1  # denominator stationary B: head B sum -> partition 1
    in_maps = []
    for core in range(8):
        b, qh = core // 2, core % 2
        xb = x[b]
        # My query half first; key/value order is permutation-invariant.
        xrot = np.concatenate([xb[qh * TQ : (qh + 1) * TQ], xb[(1 - qh) * TQ : (2 - qh) * TQ]], axis=0)
        xT_np = np.ascontiguousarray(xrot.T).astype(BF)
        in_maps.append(
            {
                "xT": xT_np,
                "wkT": wkT,
                "wqT": wqT,
                "wvT": wvT,
                "woT": woT,
                "bo": bo,
                "ones_in": ones_np,
            }
        )
    return in_maps


def gather_out(results):
    out = np.empty((B, N, C), dtype=np.float32)
    for core in range(8):
        b, qh = core // 2, core % 2
        out[b, qh * TQ : (qh + 1) * TQ, :] = results[core]["outT"].T
    return out


def kernel(x, Wq, Wk, Wv, Wo, bo):
    from concourse.bass_utils import run_bass_kernel_spmd

    in_maps = make_in_maps(x, Wq, Wk, Wv, Wo, bo)
    res = run_bass_kernel_spmd(_get_nc(), in_maps, core_ids=list(range(8)))
    return gather_out(res.results)


# revision 11
# speedup vs baseline: 1.0577x; 1.0577x over previous
"""Trainium2 Bass kernel for multi-head attention (B=4, N=2048, C=1024, H=16).

Sharding: 8 cores = (batch b, query-half qh). Each core computes attention for
its 1024 query tokens of batch b against all 2048 keys of batch b, all 16
heads, plus the output projection. Host-side work is layout only (transpose /
concat); all FLOPs run on device.

Per-core layout: activations are feature-major ("xT" = [C, tokens]) so every
matmul contracts over the partition axis. Scores are computed transposed
(ST[j keys, i queries]) which makes softmax denominators cheap and makes P@V
need no transpose of P. Softmax skips max-subtraction (|S| <~ 25 for this
distribution, exp is safe in fp32). All matmul operands are bf16.

PE array tiling: the score matmuls contract over D=64, so the two heads of a
pair run as concurrent 64x128 row tiles (T0/T8, auto-derived from the base
partitions). The AV matmuls have 64 stationary columns per head, so the two
heads run as concurrent 128x64 column tiles (T0/T1) accumulating into one
[128,512] PSUM bank - head A dims on partitions 0:64, head B on 64:128,
which is exactly the O^T layout the output projection wants. The softmax
denominator (which previously rode along as a 65th ones-column of V, blocking
column tiling) is instead accumulated per-jt on the DVE (bf16 adds at 2x
rate) and collapsed across partitions by a single ones-stationary matmul per
(pair, query-block).

Matmuls that share a stationary operand are emitted back-to-back and a
post-schedule pass drops the redundant Ldweights (walrus emits one per
matmul unconditionally).
"""

import sys

import ml_dtypes
import numpy as np

BF = ml_dtypes.bfloat16

sys.path.insert(0, "/opt/trn_rl_repo")

import concourse.bass as bass  # noqa: E402
import concourse.bacc as bacc  # noqa: E402
import concourse.mybir as mybir  # noqa: E402
import concourse.tile as tile  # noqa: E402

# Problem constants (hardcoded by contract).
B, N, C = 4, 2048, 1024
H, D = 16, 64
SCALE = D**-0.5  # 0.125
T = N  # key tokens per core
TQ = N // 2  # query tokens per core
KC = C // 128  # contraction chunks of 128
PAIRS = H // 2  # head pairs; pair g = heads (2g, 2g+1)
NJT = T // 128  # 16 key tiles
F32 = mybir.dt.float32
BF16 = mybir.dt.bfloat16
EXP = mybir.ActivationFunctionType.Exp

_KEEP_PE = ("Matmult", "EventSemaphore", "Ldweights")


def _dedupe_ldweights(nc):
    """Drop Ldweights that reload the exact weights already resident.

    Runs after tile scheduling (program order fixed, waits materialized) and
    before finalize. An Ldweights is redundant iff it has the same source AP /
    tile position as the previous kept one, carries no waits or updates of its
    own, and its dependencies are a subset of the kept load's. Only Matmult
    (non-self-loading for bf16) and EventSemaphore may sit in between; any
    other PE instruction conservatively resets the tracked state.
    """
    removed = 0
    for b in nc.main_func.blocks:
        newlist = []
        last = None
        for i in b.instructions:
            if i.opcode == "Ldweights":
                ap = i.ins[0]
                sig = (
                    ap.memsetref,
                    ap.memref,
                    ap.offset,
                    str(ap.ap),
                    str(ap.dtype),
                    str(i.tile_position),
                    str(i.perf_mode),
                    str(getattr(i, "is_transpose", None)),
                )
                deps = {n for n, _ in i.sync_dependencies()}
                deps |= {n for n, _ in i.nosync_dependencies()}
                has_wait = bool(i.sync_info and i.sync_info.on_wait)
                has_upd = bool(i.sync_info and i.sync_info.on_update)
                if (
                    last is not None
                    and sig == last[0]
                    and not has_wait
                    and not has_upd
                    and deps <= last[1]
                ):
                    removed += 1
                    continue
                last = (sig, deps)
            elif i.engine == mybir.EngineType.PE and i.opcode not in _KEEP_PE:
                last = None
            newlist.append(i)
        b.instructions[:] = newlist
    return removed


def build_bass(reps=1, loop_iters=0, ablate=()):
    from contextlib import ExitStack

    nc = bacc.Bacc()
    xT = nc.dram_tensor("xT", [C, T], BF16, kind="ExternalInput")
    wkT = nc.dram_tensor("wkT", [PAIRS, 128, KC, 128], BF16, kind="ExternalInput")
    wqT = nc.dram_tensor("wqT", [PAIRS, 128, KC, 128], BF16, kind="ExternalInput")
    wvT = nc.dram_tensor("wvT", [128, KC, 4, 256], BF16, kind="ExternalInput")
    woT = nc.dram_tensor("woT", [128, KC, C], BF16, kind="ExternalInput")
    bo = nc.dram_tensor("bo", [C], F32, kind="ExternalInput")
    # ones_in[:, 0] = 1 and ones_in[:, 65] = 1, zeros elsewhere: slices
    # [:, 0:64] / [:, 64:128] are the denominator matmul stationaries that
    # place head A's / head B's key-sum on output partitions 0 / 1.
    ones_in = nc.dram_tensor("ones_in", [128, 128], BF16, kind="ExternalInput")
    outT = nc.dram_tensor("outT", [C, TQ], F32, kind="ExternalOutput")

    xT_r = xT.rearrange("(kc p) t -> p kc t", p=128)
    bo_r = bo.rearrange("(a p) -> p a", p=128)
    outT_r = outT.rearrange("(et p) i -> et p i", p=128)

    with tile.TileContext(nc) as tc, ExitStack() as ctx:
        ctx.enter_context(
            nc.allow_low_precision(reason="bf16 matmul path is within error budget")
        )
        const = ctx.enter_context(tc.tile_pool(name="const", bufs=1))
        wpool = ctx.enter_context(tc.tile_pool(name="wpool", bufs=2))
        wvp = ctx.enter_context(tc.tile_pool(name="wvp", bufs=1))
        kqp = ctx.enter_context(tc.tile_pool(name="kqp", bufs=2))
        vpool = ctx.enter_context(tc.tile_pool(name="vpool", bufs=1))
        # pt tiles live for a full block (16 jt) plus the production margin of
        # the next block: the consume side (AV + denominator adds) lags one
        # whole (pair, query-block) behind the produce side (ST + exp) so no
        # consumer ever waits on a just-produced tile.
        ptp = ctx.enter_context(tc.tile_pool(name="ptp", bufs=NJT + 4))
        accp = ctx.enter_context(tc.tile_pool(name="accp", bufs=2))
        smp = ctx.enter_context(tc.tile_pool(name="smp", bufs=2))
        obp = ctx.enter_context(tc.tile_pool(name="obp", bufs=2))
        # PSUM budget (8 banks): st2 2x[128,2,512] = 4, av 1, dn 1, proj 2.
        pp = ctx.enter_context(tc.tile_pool(name="pp", bufs=2, space="PSUM"))
        psA = ctx.enter_context(tc.tile_pool(name="psA", bufs=1, space="PSUM"))
        ps2 = ctx.enter_context(tc.tile_pool(name="ps2", bufs=2, space="PSUM"))
        drp = ctx.enter_context(tc.tile_pool(name="drp", bufs=2, space="DRAM"))

        import contextlib
        loop_ctx = (
            tc.For_i(0, loop_iters, 1) if loop_iters else contextlib.nullcontext()
        )
        with loop_ctx:
          for _rep in range(reps):
            # Constants first (small), then xT per c-chunk so the first
            # projection matmuls start as soon as chunk 0 lands.
            ones = const.tile([128, 128], BF16, tag="ones")
            nc.sync.dma_start(out=ones, in_=ones_in[:, :])
            bo_t = const.tile([128, KC], F32, tag="bo")
            nc.sync.dma_start(out=bo_t, in_=bo_r)
            xt = const.tile([128, KC, T], BF16, tag="xw")
            # O^T, concatenated over heads: rows fc*128+p = feature f, cols = query i.
            ot_t = const.tile([128, PAIRS, TQ], BF16, tag="ot")

            def kq_alloc(g):
                """Allocate tiles and start weight DMAs for pair g's k/q projections."""
                t = {}
                t["wk"] = wpool.tile([128, KC, 128], BF16, tag="wk", name=f"wk{g}")
                nc.sync.dma_start(out=t["wk"], in_=wkT[g])
                t["wq"] = wpool.tile([128, KC, 128], BF16, tag="wq", name=f"wq{g}")
                nc.sync.dma_start(out=t["wq"], in_=wqT[g])
                t["kt"] = kqp.tile([128, T], BF16, tag="kT", name=f"kt{g}")
                t["qt"] = kqp.tile([128, TQ], BF16, tag="qT", name=f"qt{g}")
                return t

            def v_emit_all(wv_t, v_t):
                """V projection for all 4 head-groups: per (tt, kc) one xt
                stationary serves 4 back-to-back matmuls (dedupe drops 3
                Ldweights). PSUM evacuation runs on the (otherwise idle)
                scalar engine."""
                if "proj" in ablate:
                    nc.vector.tensor_copy(v_t[:, 0, 0:4, :], xt[:, 0, 0:256])
                    yield
                    return
                for tt in range(NJT):
                    pvs = [
                        pp.tile([128, 256], F32, tag="ps", name=f"pv{tt}_0"),
                        pp.tile([128, 256], F32, tag="ps", name=f"pv{tt}_1"),
                        psA.tile([128, 256], F32, tag="av", name=f"pv{tt}_2"),
                        psA.tile([128, 256], F32, tag="dn", name=f"pv{tt}_3"),
                    ]
                    for kc in range(KC):
                        for p in range(4):
                            nc.tensor.matmul(
                                pvs[p],
                                xt[:, kc, tt * 128 : (tt + 1) * 128],
                                wv_t[:, kc, p, :],
                                start=(kc == 0),
                                stop=(kc == KC - 1),
                            )
                        yield
                    for p in range(4):
                        nc.scalar.copy(v_t[:, tt, 4 * p : 4 * p + 4, :], pvs[p])
                    yield

            def kq_emit(g, t):
                if "proj" in ablate:
                    nc.vector.tensor_copy(t["kt"][:, 0:512], xt[:, 0, 0:512])
                    nc.vector.tensor_copy(t["qt"][:, 0:512], xt[:, 1, 0:512])
                    yield
                    return
                kt_t = t["kt"]
                # Token-pair groups: one wk stationary serves 2 adjacent
                # matmuls (dedupe drops every other Ldweights).
                for tcn2 in range(T // 1024):
                    pks = [
                        pp.tile([128, 512], F32, tag="ps", name=f"pk{g}_{tcn2}_{h}")
                        for h in range(2)
                    ]
                    for kc in range(KC):
                        for h in range(2):
                            tcn = tcn2 * 2 + h
                            nc.tensor.matmul(
                                pks[h],
                                t["wk"][:, kc, :],
                                xt[:, kc, tcn * 512 : (tcn + 1) * 512],
                                start=(kc == 0),
                                stop=(kc == KC - 1),
                            )
                        yield
                    for h in range(2):
                        tcn = tcn2 * 2 + h
                        nc.vector.tensor_copy(
                            kt_t[:, tcn * 512 : (tcn + 1) * 512], pks[h]
                        )
                        yield
                qt_t = t["qt"]
                pqs = [
                    pp.tile([128, 512], F32, tag="ps", name=f"pq{g}_{icn}")
                    for icn in range(TQ // 512)
                ]
                for kc in range(KC):
                    for icn in range(TQ // 512):
                        nc.tensor.matmul(
                            pqs[icn],
                            t["wq"][:, kc, :],
                            xt[:, kc, icn * 512 : (icn + 1) * 512],
                            start=(kc == 0),
                            stop=(kc == KC - 1),
                        )
                    yield
                for icn in range(TQ // 512):
                    nc.vector.tensor_copy(
                        qt_t[:, icn * 512 : (icn + 1) * 512], pqs[icn]
                    )
                    yield

            # V projection for all groups runs up front. Pair 0's k/q weight
            # DMAs start first so they land before the V phase ends.
            tiles = [None] * PAIRS
            tiles[0] = kq_alloc(0)
            wv_t = wvp.tile([128, KC, 4, 256], BF16, tag="wv")
            nc.sync.dma_start(out=wv_t, in_=wvT[:, :, :, :])
            # V per head, 64-wide (no ones column): [p, jt, head, d].
            v_all = vpool.tile([128, NJT, H, 64], BF16, tag="v")
            for kc in range(KC):
                nc.sync.dma_start(out=xt[:, kc, :], in_=xT_r[:, kc, :])
            for _ in v_emit_all(wv_t, v_all):
                pass
            for _ in kq_emit(0, tiles[0]):
                pass

            def emit_tail(g, isl, av, acc):
                """Denominator + evacuation for one finished (pair, query-block).

                Emitted at the end of the block AFTER the one it belongs to
                (the consume side lags a full block), so acc and av are long
                done. Returns a closure for the final normalize mul, to be
                invoked a couple of steps later still - by then the
                reciprocal's DRAM-broadcast bounce has landed, so the DVE
                never blocks on it.
                """
                if "noepi" in ablate:
                    av_sb = smp.tile([128, 512], BF16, tag="avs", name=f"as{g}_{isl.start}")
                    nc.vector.tensor_copy(av_sb, av)
                    nc.vector.tensor_copy(ot_t[:, g, isl], av_sb)
                    return lambda: None
                # Collapse acc across the 128 key partitions with
                # ones-stationary matmuls; head A lands on output partition 0,
                # head B on partition 1 (128x64 mode, same as AV).
                dn = psA.tile([64, 512], F32, tag="dn", name=f"dn{g}_{isl.start}")
                nc.tensor.matmul(dn, ones[:, 0:64], acc[:, 0, :],
                                 start=True, stop=False)
                nc.tensor.matmul(dn, ones[:, 64:128], acc[:, 1, :],
                                 start=False, stop=True)
                # Copy accumulator to SBUF right away to free the av bank.
                av_sb = smp.tile([128, 512], BF16, tag="avs", name=f"as{g}_{isl.start}")
                nc.vector.tensor_copy(av_sb, av)
                rec = smp.tile([2, 512], BF16, tag="rec", name=f"rc{g}_{isl.start}")
                nc.vector.reciprocal(rec, dn[0:2, :])
                # Bounce through DRAM and broadcast-load across partitions
                # (stride-0 DRAM AP): rows 0:64 get 1/dA, 64:128 get 1/dB.
                rec_d = drp.tile([2, 512], BF16, tag="rd", name=f"rd{g}_{isl.start}")
                nc.sync.dma_start(out=rec_d, in_=rec)
                bc = smp.tile([128, 512], BF16, tag="bc", name=f"bc{g}_{isl.start}")
                nc.sync.dma_start(
                    out=bc[0:64, :],
                    in_=bass.AP(tensor=rec_d[:, :].tensor, offset=rec_d[0:1, :].offset,
                                ap=[[0, 64], [1, 512]]),
                )
                nc.sync.dma_start(
                    out=bc[64:128, :],
                    in_=bass.AP(tensor=rec_d[:, :].tensor, offset=rec_d[1:2, :].offset,
                                ap=[[0, 64], [1, 512]]),
                )

                def mul():
                    # Normalize both heads straight into O^T layout.
                    nc.vector.tensor_mul(ot_t[:, g, isl], av_sb, bc)

                return mul

            def consume_step(blk, av, acc, jt):
                """One lagged consume step: AV matmuls + denominator add for
                key-tile jt of the previous (pair, query-block)."""
                pg = blk["g"]
                pv = blk["pts"][jt]
                if "noav" not in ablate:
                    nc.tensor.matmul(
                        av[0:64, :], v_all[:, jt, 2 * pg, :], pv[:, 0, :],
                        start=(jt == 0), stop=(jt == NJT - 1),
                        skip_group_check=True,
                    )
                    nc.tensor.matmul(
                        av[64:128, :], v_all[:, jt, 2 * pg + 1, :], pv[:, 1, :],
                        start=(jt == 0), stop=(jt == NJT - 1),
                        skip_group_check=True,
                    )
                elif jt == 0:
                    nc.vector.tensor_copy(av[0:2, 0:2], pv[0:2, 0, 0:2])
                if "nodve" in ablate:
                    if jt == 0:
                        nc.vector.tensor_copy(acc[:, 0, 0:2], pv[:, 0, 0:2])
                elif jt == 0:
                    nc.vector.tensor_copy(acc, pv)
                else:
                    nc.vector.tensor_add(acc, acc, pv)

            # The attention pipeline: the produce side of block k (scores +
            # exp) runs while the consume side (AV accumulation, denominator
            # adds, epilogue) works on block k-1. Every cross-engine
            # dependency is therefore ~16 steps stale and no engine queue
            # ever blocks another.
            prev_blk = None
            pending_mul = None
            pending_tail = None
            for g in range(PAIRS):
                t = tiles[g]
                kt_t, qt_t = t["kt"], t["qt"]
                if g + 1 < PAIRS:
                    tiles[g + 1] = kq_alloc(g + 1)
                    kq_gen = kq_emit(g + 1, tiles[g + 1])
                else:
                    kq_gen = iter(())

                if "att" in ablate:
                    for _ in kq_gen:
                        pass
                    nc.vector.tensor_copy(ot_t[0:64, g, 0:512], kt_t[0:64, 0:512])
                    continue
                for icn in range(TQ // 512):
                    isl = slice(icn * 512, (icn + 1) * 512)
                    if prev_blk is not None:
                        # O^T accumulator for the PREVIOUS block, both heads
                        # via column tiling: head A dims on partitions 0:64
                        # (tile T0), head B on 64:128 (tile T1). One bank.
                        av = psA.tile([128, 512], F32, tag="av", name=f"av{g}_{icn}")
                        acc = accp.tile([128, 2, 512], BF16, tag="acc", name=f"ac{g}_{icn}")
                    pts = []
                    # 2-jt macro steps: group same-tile-mode matmuls (4 score
                    # MMs, then projections, then 4 AV MMs) so the PE array
                    # re-tiles 3 times per TWO key-tiles instead of per one.
                    for jm in range(NJT // 2):
                        sts = []
                        for h in range(2):
                            jt = 2 * jm + h
                            jsl = slice(jt * 128, (jt + 1) * 128)
                            # S^T[j, i] for both heads into one 2-bank PSUM
                            # tile; heads run as concurrent 64x128 row tiles.
                            st2 = ps2.tile([128, 2, 512], F32, tag="st2", name=f"st{g}_{icn}_{jt}")
                            nc.tensor.matmul(st2[:, 0, :], kt_t[0:64, jsl], qt_t[0:64, isl])
                            nc.tensor.matmul(st2[:, 1, :], kt_t[64:128, jsl], qt_t[64:128, isl])
                            sts.append(st2)
                        for h in range(2):
                            jt = 2 * jm + h
                            # One exp instruction covers both heads (1024 free).
                            pt2 = ptp.tile([128, 2, 512], BF16, tag="pt", name=f"pt{g}_{icn}_{jt}")
                            if "noact" in ablate:
                                nc.scalar.copy(pt2[:, 0, 0:2], sts[h][:, 0, 0:2])
                            else:
                                nc.scalar.activation(pt2[:, :, :], sts[h][:, :, :], EXP, scale=SCALE)
                            pts.append(pt2)
                        # Interleave next pair's projections while ACT runs
                        # (~2 proj MMs per key-tile; each yield emits two
                        # matmuls). More during the fill block, which has no
                        # consume work.
                        for _ in range(2 if prev_blk is not None else 8):
                            next(kq_gen, None)
                        if prev_blk is not None:
                            consume_step(prev_blk, av, acc, 2 * jm)
                            consume_step(prev_blk, av, acc, 2 * jm + 1)
                        if jm == 1 and pending_mul is not None:
                            pending_mul()
                            pending_mul = None
                    if prev_blk is not None:
                        pending_mul = emit_tail(prev_blk["g"], prev_blk["isl"], av, acc)
                    prev_blk = {"g": g, "isl": isl, "pts": pts}
                for _ in kq_gen:
                    pass
            if "att" not in ablate:
                # Drain: consume the final block (no produce side left).
                av = psA.tile([128, 512], F32, tag="av", name="av_drain")
                acc = accp.tile([128, 2, 512], BF16, tag="acc", name="ac_drain")
                for jm in range(NJT // 2):
                    consume_step(prev_blk, av, acc, 2 * jm)
                    consume_step(prev_blk, av, acc, 2 * jm + 1)
                    if jm == 1 and pending_mul is not None:
                        pending_mul()
                        pending_mul = None
                emit_tail(prev_blk["g"], prev_blk["isl"], av, acc)()

            if "out" in ablate:
                ob0 = obp.tile([128, 512], F32, tag="ob", name="ob0")
                nc.vector.tensor_copy(ob0, ot_t[:, 0, 0:512])
                nc.sync.dma_start(out=outT_r[0, :, 0:512], in_=ob0)
                continue
            # Output projection: outT[e, i] = Wo @ O^T + bo. Both query halves
            # share each wo stationary (adjacent matmuls, dedupe).
            wo_t = const.tile([128, KC, C], BF16, tag="xw")
            nc.sync.dma_start(out=wo_t, in_=woT[:, :, :])
            for et in range(C // 128):
                pos = [
                    pp.tile([128, 512], F32, tag="ps", name=f"po{et}_{icn}")
                    for icn in range(TQ // 512)
                ]
                for fc in range(KC):
                    for icn in range(TQ // 512):
                        nc.tensor.matmul(
                            pos[icn],
                            wo_t[:, fc, et * 128 : (et + 1) * 128],
                            ot_t[:, fc, icn * 512 : (icn + 1) * 512],
                            start=(fc == 0),
                            stop=(fc == KC - 1),
                        )
                for icn in range(TQ // 512):
                    ob = obp.tile([128, 512], F32, tag="ob", name=f"o{et}_{icn}")
                    nc.vector.tensor_scalar_add(ob, pos[icn], bo_t[:, et : et + 1])
                    nc.sync.dma_start(
                        out=outT_r[et, :, icn * 512 : (icn + 1) * 512], in_=ob
                    )

    _dedupe_ldweights(nc)
    nc.finalize()
    return nc


_CACHE = {}


def _get_nc():
    if "nc" not in _CACHE:
        _CACHE["nc"] = build_bass()
    return _CACHE["nc"]


def make_in_maps(x, Wq, Wk, Wv, Wo, bo):
    """Host-side sharding: layout prep only (transposes / concatenation)."""
    x = np.asarray(x, dtype=np.float32)
    # Weights pre-tiled into the exact SBUF layouts (contiguous DMAs).
    # wk/wq: [g, p, kc, o] = W[g*128+o, kc*128+p]
    wkT = np.ascontiguousarray(
        np.asarray(Wk, np.float32).reshape(PAIRS, 128, KC, 128).transpose(0, 3, 2, 1)
    ).astype(BF)
    wqT = np.ascontiguousarray(
        np.asarray(Wq, np.float32).reshape(PAIRS, 128, KC, 128).transpose(0, 3, 2, 1)
    ).astype(BF)
    # wv: [p, kc, grp, col] where grp covers 4 heads x 64 dims = 256 cols
    wvT = np.ascontiguousarray(
        np.asarray(Wv, np.float32).T.reshape(KC, 128, 4, 256).transpose(1, 0, 2, 3)
    ).astype(BF)
    # wo: [p, fc, e] = Wo[e, fc*128+p]
    woT = np.ascontiguousarray(
        np.asarray(Wo, np.float32).T.reshape(KC, 128, C).transpose(1, 0, 2)
    ).astype(BF)
    bo = np.ascontiguousarray(np.asarray(bo, np.float32))
    ones_np = np.zeros((128, 128), BF)
    ones_np[:, 0] = 1  # denominator stationary A: head A sum -> partition 0
    ones_np[:, 65] = 1  # denominator stationary B: head B sum -> partition 1
    in_maps = []
    for core in range(8):
        b, qh = core // 2, core % 2
        xb = x[b]
        # My query half first; key/value order is permutation-invariant.
        xrot = np.concatenate([xb[qh * TQ : (qh + 1) * TQ], xb[(1 - qh) * TQ : (2 - qh) * TQ]], axis=0)
        xT_np = np.ascontiguousarray(xrot.T).astype(BF)
        in_maps.append(
            {
                "xT": xT_np,
                "wkT": wkT,
                "wqT": wqT,
                "wvT": wvT,
                "woT": woT,
                "bo": bo,
                "ones_in": ones_np,
            }
        )
    return in_maps


def gather_out(results):
    out = np.empty((B, N, C), dtype=np.float32)
    for core in range(8):
        b, qh = core // 2, core % 2
        out[b, qh * TQ : (qh + 1) * TQ, :] = results[core]["outT"].T
    return out


def kernel(x, Wq, Wk, Wv, Wo, bo):
    from concourse.bass_utils import run_bass_kernel_spmd

    in_maps = make_in_maps(x, Wq, Wk, Wv, Wo, bo)
    res = run_bass_kernel_spmd(_get_nc(), in_maps, core_ids=list(range(8)))
    return gather_out(res.results)


# revision 13
# speedup vs baseline: 1.0996x; 1.0397x over previous
"""Trainium2 Bass kernel for multi-head attention (B=4, N=2048, C=1024, H=16).

Sharding: 8 cores = (batch b, query-half qh). Each core computes attention for
its 1024 query tokens of batch b against all 2048 keys of batch b, all 16
heads, plus the output projection. Host-side work is layout only (transpose /
concat); all FLOPs run on device.

Per-core layout: activations are feature-major ("xT" = [C, tokens]) so every
matmul contracts over the partition axis. Scores are computed transposed
(ST[j keys, i queries]) which makes softmax denominators cheap and makes P@V
need no transpose of P. Softmax skips max-subtraction (|S| <~ 25 for this
distribution, exp is safe in fp32). All matmul operands are bf16.

PE array tiling: the score matmuls contract over D=64, so the two heads of a
pair run as concurrent 64x128 row tiles (T0/T8, auto-derived from the base
partitions). The AV matmuls have 64 stationary columns per head, so the two
heads run as concurrent 128x64 column tiles (T0/T1) accumulating into one
[128,512] PSUM bank - head A dims on partitions 0:64, head B on 64:128,
which is exactly the O^T layout the output projection wants. The softmax
denominator (which previously rode along as a 65th ones-column of V, blocking
column tiling) is instead accumulated per-jt on the DVE (bf16 adds at 2x
rate) and collapsed across partitions by a single ones-stationary matmul per
(pair, query-block).

Matmuls that share a stationary operand are emitted back-to-back and a
post-schedule pass drops the redundant Ldweights (walrus emits one per
matmul unconditionally).
"""

import sys

import ml_dtypes
import numpy as np

BF = ml_dtypes.bfloat16

sys.path.insert(0, "/opt/trn_rl_repo")

import concourse.bass as bass  # noqa: E402
import concourse.bacc as bacc  # noqa: E402
import concourse.mybir as mybir  # noqa: E402
import concourse.tile as tile  # noqa: E402

# Problem constants (hardcoded by contract).
B, N, C = 4, 2048, 1024
H, D = 16, 64
SCALE = D**-0.5  # 0.125
T = N  # key tokens per core
TQ = N // 2  # query tokens per core
KC = C // 128  # contraction chunks of 128
PAIRS = H // 2  # head pairs; pair g = heads (2g, 2g+1)
NJT = T // 128  # 16 key tiles
F32 = mybir.dt.float32
BF16 = mybir.dt.bfloat16
EXP = mybir.ActivationFunctionType.Exp

_KEEP_PE = ("Matmult", "EventSemaphore", "Ldweights")


def _dedupe_ldweights(nc):
    """Drop Ldweights that reload the exact weights already resident.

    Runs after tile scheduling (program order fixed, waits materialized) and
    before finalize. An Ldweights is redundant iff it has the same source AP /
    tile position as the previous kept one, carries no waits or updates of its
    own, and its dependencies are a subset of the kept load's. Only Matmult
    (non-self-loading for bf16) and EventSemaphore may sit in between; any
    other PE instruction conservatively resets the tracked state.
    """
    removed = 0
    for b in nc.main_func.blocks:
        newlist = []
        last = None
        for i in b.instructions:
            if i.opcode == "Ldweights":
                ap = i.ins[0]
                sig = (
                    ap.memsetref,
                    ap.memref,
                    ap.offset,
                    str(ap.ap),
                    str(ap.dtype),
                    str(i.tile_position),
                    str(i.perf_mode),
                    str(getattr(i, "is_transpose", None)),
                )
                deps = {n for n, _ in i.sync_dependencies()}
                deps |= {n for n, _ in i.nosync_dependencies()}
                has_wait = bool(i.sync_info and i.sync_info.on_wait)
                has_upd = bool(i.sync_info and i.sync_info.on_update)
                if (
                    last is not None
                    and sig == last[0]
                    and not has_wait
                    and not has_upd
                    and deps <= last[1]
                ):
                    removed += 1
                    continue
                last = (sig, deps)
            elif i.engine == mybir.EngineType.PE and i.opcode not in _KEEP_PE:
                last = None
            newlist.append(i)
        b.instructions[:] = newlist
    return removed


def build_bass(reps=1, loop_iters=0, ablate=()):
    from contextlib import ExitStack

    nc = bacc.Bacc()
    xT = nc.dram_tensor("xT", [C, T], BF16, kind="ExternalInput")
    wkT = nc.dram_tensor("wkT", [PAIRS, 128, KC, 128], BF16, kind="ExternalInput")
    wqT = nc.dram_tensor("wqT", [PAIRS, 128, KC, 128], BF16, kind="ExternalInput")
    wvT = nc.dram_tensor("wvT", [128, KC, 4, 256], BF16, kind="ExternalInput")
    woT = nc.dram_tensor("woT", [128, KC, C], BF16, kind="ExternalInput")
    bo = nc.dram_tensor("bo", [C], F32, kind="ExternalInput")
    # ones_in[:, 0] = 1 and ones_in[:, 65] = 1, zeros elsewhere: slices
    # [:, 0:64] / [:, 64:128] are the denominator matmul stationaries that
    # place head A's / head B's key-sum on output partitions 0 / 1.
    ones_in = nc.dram_tensor("ones_in", [128, 128], BF16, kind="ExternalInput")
    outT = nc.dram_tensor("outT", [C, TQ], F32, kind="ExternalOutput")

    xT_r = xT.rearrange("(kc p) t -> p kc t", p=128)
    bo_r = bo.rearrange("(a p) -> p a", p=128)
    outT_r = outT.rearrange("(et p) i -> et p i", p=128)

    with tile.TileContext(nc) as tc, ExitStack() as ctx:
        ctx.enter_context(
            nc.allow_low_precision(reason="bf16 matmul path is within error budget")
        )
        const = ctx.enter_context(tc.tile_pool(name="const", bufs=1))
        wpool = ctx.enter_context(tc.tile_pool(name="wpool", bufs=2))
        wvp = ctx.enter_context(tc.tile_pool(name="wvp", bufs=1))
        kqp = ctx.enter_context(tc.tile_pool(name="kqp", bufs=2))
        vpool = ctx.enter_context(tc.tile_pool(name="vpool", bufs=1))
        # pt tiles live for a full block (16 jt) plus the production margin of
        # the next block: the consume side (AV + denominator adds) lags one
        # whole (pair, query-block) behind the produce side (ST + exp) so no
        # consumer ever waits on a just-produced tile.
        ptp = ctx.enter_context(tc.tile_pool(name="ptp", bufs=NJT + 4))
        accp = ctx.enter_context(tc.tile_pool(name="accp", bufs=2))
        smp = ctx.enter_context(tc.tile_pool(name="smp", bufs=2))
        obp = ctx.enter_context(tc.tile_pool(name="obp", bufs=2))
        # PSUM budget (8 banks): st2 2x[128,2,512] = 4, av 1, dn 1, proj 2.
        pp = ctx.enter_context(tc.tile_pool(name="pp", bufs=2, space="PSUM"))
        psA = ctx.enter_context(tc.tile_pool(name="psA", bufs=1, space="PSUM"))
        ps2 = ctx.enter_context(tc.tile_pool(name="ps2", bufs=2, space="PSUM"))
        drp = ctx.enter_context(tc.tile_pool(name="drp", bufs=2, space="DRAM"))

        import contextlib
        loop_ctx = (
            tc.For_i(0, loop_iters, 1) if loop_iters else contextlib.nullcontext()
        )
        with loop_ctx:
          for _rep in range(reps):
            # Constants first (small), then xT per c-chunk so the first
            # projection matmuls start as soon as chunk 0 lands.
            ones = const.tile([128, 128], BF16, tag="ones")
            nc.sync.dma_start(out=ones, in_=ones_in[:, :])
            bo_t = const.tile([128, KC], F32, tag="bo")
            nc.sync.dma_start(out=bo_t, in_=bo_r)
            xt = const.tile([128, KC, T], BF16, tag="xw")
            # O^T, concatenated over heads: rows fc*128+p = feature f, cols = query i.
            ot_t = const.tile([128, PAIRS, TQ], BF16, tag="ot")

            def kq_alloc(g):
                """Allocate tiles and start weight DMAs for pair g's k/q projections."""
                t = {}
                t["wk"] = wpool.tile([128, KC, 128], BF16, tag="wk", name=f"wk{g}")
                nc.sync.dma_start(out=t["wk"], in_=wkT[g])
                t["wq"] = wpool.tile([128, KC, 128], BF16, tag="wq", name=f"wq{g}")
                nc.sync.dma_start(out=t["wq"], in_=wqT[g])
                t["kt"] = kqp.tile([128, T], BF16, tag="kT", name=f"kt{g}")
                t["qt"] = kqp.tile([128, TQ], BF16, tag="qT", name=f"qt{g}")
                return t

            def v_emit_front(wv_t, v_t, p):
                """V projection for head-group p in the prologue: PSUM from
                the pp pool, evacuation on the (idle) scalar engine."""
                if "proj" in ablate:
                    nc.vector.tensor_copy(v_t[:, 0, 0:4, :], xt[:, 0, 0:256])
                    return
                for tt2 in range(NJT // 2):
                    pvs = [
                        pp.tile([128, 256], F32, tag="ps", name=f"pv{tt2}_{h}")
                        for h in range(2)
                    ]
                    for kc in range(KC):
                        for h in range(2):
                            tt = 2 * tt2 + h
                            nc.tensor.matmul(
                                pvs[h],
                                xt[:, kc, tt * 128 : (tt + 1) * 128],
                                wv_t[:, kc, p, :],
                                start=(kc == 0),
                                stop=(kc == KC - 1),
                            )
                    for h in range(2):
                        tt = 2 * tt2 + h
                        nc.scalar.copy(
                            v_t[:, tt, 4 * (p % 2) : 4 * (p % 2) + 4, :], pvs[h]
                        )

            def v_emit_bg(wv_t, v_t, p):
                """V projection for head-group p, interleaved into attention
                slack: single-PSUM units via the dn-tag bank, evacuation on
                the DVE (the scalar engine is saturated by exp there)."""
                if "proj" in ablate:
                    return
                for tt in range(NJT):
                    # One pull = one whole unit (8 accumulating MMs + copy):
                    # a unit must never straddle a block boundary, where
                    # emit_tail allocates the same dn-tag PSUM slab - a split
                    # would deadlock the in-order PE queue on the ring.
                    pv = psA.tile([128, 256], F32, tag="dn", name=f"pvb{p}_{tt}")
                    for kc in range(KC):
                        nc.tensor.matmul(
                            pv,
                            xt[:, kc, tt * 128 : (tt + 1) * 128],
                            wv_t[:, kc, p, :],
                            start=(kc == 0),
                            stop=(kc == KC - 1),
                        )
                    nc.vector.tensor_copy(
                        v_t[:, tt, 4 * (p % 2) : 4 * (p % 2) + 4, :], pv
                    )
                    yield

            def kq_emit(g, t):
                if "proj" in ablate:
                    nc.vector.tensor_copy(t["kt"][:, 0:512], xt[:, 0, 0:512])
                    nc.vector.tensor_copy(t["qt"][:, 0:512], xt[:, 1, 0:512])
                    yield
                    return
                kt_t = t["kt"]
                # Token-pair groups: one wk stationary serves 2 adjacent
                # matmuls (dedupe drops every other Ldweights).
                for tcn2 in range(T // 1024):
                    pks = [
                        pp.tile([128, 512], F32, tag="ps", name=f"pk{g}_{tcn2}_{h}")
                        for h in range(2)
                    ]
                    for kc in range(KC):
                        for h in range(2):
                            tcn = tcn2 * 2 + h
                            nc.tensor.matmul(
                                pks[h],
                                t["wk"][:, kc, :],
                                xt[:, kc, tcn * 512 : (tcn + 1) * 512],
                                start=(kc == 0),
                                stop=(kc == KC - 1),
                            )
                        yield
                    for h in range(2):
                        tcn = tcn2 * 2 + h
                        nc.vector.tensor_copy(
                            kt_t[:, tcn * 512 : (tcn + 1) * 512], pks[h]
                        )
                        yield
                qt_t = t["qt"]
                pqs = [
                    pp.tile([128, 512], F32, tag="ps", name=f"pq{g}_{icn}")
                    for icn in range(TQ // 512)
                ]
                for kc in range(KC):
                    for icn in range(TQ // 512):
                        nc.tensor.matmul(
                            pqs[icn],
                            t["wq"][:, kc, :],
                            xt[:, kc, icn * 512 : (icn + 1) * 512],
                            start=(kc == 0),
                            stop=(kc == KC - 1),
                        )
                    yield
                for icn in range(TQ // 512):
                    nc.vector.tensor_copy(
                        qt_t[:, icn * 512 : (icn + 1) * 512], pqs[icn]
                    )
                    yield

            # Prologue: only head-group 0 of the V projection (pairs 0-1)
            # plus pair 0's K/Q run before attention starts; V groups 1-3 are
            # interleaved into attention slack behind the K/Q projections.
            # Two separate V tiles so early pairs' AV matmuls never wait on
            # late V writes.
            tiles = [None] * PAIRS
            tiles[0] = kq_alloc(0)
            wv_t = wvp.tile([128, KC, 4, 256], BF16, tag="wv")
            nc.sync.dma_start(out=wv_t, in_=wvT[:, :, :, :])
            # V per head, 64-wide (no ones column): [p, jt, head-of-half, d].
            v01 = vpool.tile([128, NJT, H // 2, 64], BF16, tag="v01")
            v23 = vpool.tile([128, NJT, H // 2, 64], BF16, tag="v23")
            for kc in range(KC):
                nc.sync.dma_start(out=xt[:, kc, :], in_=xT_r[:, kc, :])
            v_emit_front(wv_t, v01, 0)
            if "proj" in ablate:
                nc.vector.tensor_copy(v23[:, 0, 0:4, :], xt[:, 0, 0:256])
            for _ in kq_emit(0, tiles[0]):
                pass
            from collections import deque
            bg = deque()
            for p in (1, 2, 3):
                bg.append(v_emit_bg(wv_t, v01 if p < 2 else v23, p))

            def bg_pull(n):
                for _ in range(n):
                    while bg:
                        try:
                            next(bg[0])
                            break
                        except StopIteration:
                            bg.popleft()
                    else:
                        return

            def emit_tail(g, isl, av, acc):
                """Denominator + evacuation for one finished (pair, query-block).

                Emitted at the end of the block AFTER the one it belongs to
                (the consume side lags a full block), so acc and av are long
                done. Returns a closure for the final normalize mul, to be
                invoked a couple of steps later still - by then the
                reciprocal's DRAM-broadcast bounce has landed, so the DVE
                never blocks on it.
                """
                if "noepi" in ablate:
                    av_sb = smp.tile([128, 512], BF16, tag="avs", name=f"as{g}_{isl.start}")
                    nc.vector.tensor_copy(av_sb, av)
                    nc.vector.tensor_copy(ot_t[:, g, isl], av_sb)
                    return lambda: None
                # Collapse acc across the 128 key partitions with
                # ones-stationary matmuls; head A lands on output partition 0,
                # head B on partition 1 (128x64 mode, same as AV).
                dn = psA.tile([64, 512], F32, tag="dn", name=f"dn{g}_{isl.start}")
                nc.tensor.matmul(dn, ones[:, 0:64], acc[:, 0, :],
                                 start=True, stop=False)
                nc.tensor.matmul(dn, ones[:, 64:128], acc[:, 1, :],
                                 start=False, stop=True)
                # Copy accumulator to SBUF right away to free the av bank.
                av_sb = smp.tile([128, 512], BF16, tag="avs", name=f"as{g}_{isl.start}")
                nc.vector.tensor_copy(av_sb, av)
                rec = smp.tile([2, 512], BF16, tag="rec", name=f"rc{g}_{isl.start}")
                nc.vector.reciprocal(rec, dn[0:2, :])
                # Bounce through DRAM and broadcast-load across partitions
                # (stride-0 DRAM AP): rows 0:64 get 1/dA, 64:128 get 1/dB.
                rec_d = drp.tile([2, 512], BF16, tag="rd", name=f"rd{g}_{isl.start}")
                nc.sync.dma_start(out=rec_d, in_=rec)
                bc = smp.tile([128, 512], BF16, tag="bc", name=f"bc{g}_{isl.start}")
                nc.sync.dma_start(
                    out=bc[0:64, :],
                    in_=bass.AP(tensor=rec_d[:, :].tensor, offset=rec_d[0:1, :].offset,
                                ap=[[0, 64], [1, 512]]),
                )
                nc.sync.dma_start(
                    out=bc[64:128, :],
                    in_=bass.AP(tensor=rec_d[:, :].tensor, offset=rec_d[1:2, :].offset,
                                ap=[[0, 64], [1, 512]]),
                )

                def mul():
                    # Normalize both heads straight into O^T layout.
                    nc.vector.tensor_mul(ot_t[:, g, isl], av_sb, bc)

                return mul

            def consume_step(blk, av, acc, jt):
                """One lagged consume step: AV matmuls + denominator add for
                key-tile jt of the previous (pair, query-block)."""
                pg = blk["g"]
                pv = blk["pts"][jt]
                v_t = v01 if pg < 4 else v23
                vh = (2 * pg) % 8
                if "noav" not in ablate:
                    nc.tensor.matmul(
                        av[0:64, :], v_t[:, jt, vh, :], pv[:, 0, :],
                        start=(jt == 0), stop=(jt == NJT - 1),
                        skip_group_check=True,
                    )
                    nc.tensor.matmul(
                        av[64:128, :], v_t[:, jt, vh + 1, :], pv[:, 1, :],
                        start=(jt == 0), stop=(jt == NJT - 1),
                        skip_group_check=True,
                    )
                elif jt == 0:
                    nc.vector.tensor_copy(av[0:2, 0:2], pv[0:2, 0, 0:2])
                if "nodve" in ablate:
                    if jt == 0:
                        nc.vector.tensor_copy(acc[:, 0, 0:2], pv[:, 0, 0:2])
                elif jt == 0:
                    nc.vector.tensor_copy(acc, pv)
                else:
                    nc.vector.tensor_add(acc, acc, pv)

            # The attention pipeline: the produce side of block k (scores +
            # exp) runs while the consume side (AV accumulation, denominator
            # adds, epilogue) works on block k-1. Every cross-engine
            # dependency is therefore ~16 steps stale and no engine queue
            # ever blocks another.
            prev_blk = None
            pending_mul = None
            pending_tail = None
            for g in range(PAIRS):
                t = tiles[g]
                kt_t, qt_t = t["kt"], t["qt"]
                if g + 1 < PAIRS:
                    tiles[g + 1] = kq_alloc(g + 1)
                    kq_gen = kq_emit(g + 1, tiles[g + 1])
                else:
                    kq_gen = iter(())

                if "att" in ablate:
                    for _ in kq_gen:
                        pass
                    nc.vector.tensor_copy(ot_t[0:64, g, 0:512], kt_t[0:64, 0:512])
                    continue
                for icn in range(TQ // 512):
                    isl = slice(icn * 512, (icn + 1) * 512)
                    if prev_blk is not None:
                        # O^T accumulator for the PREVIOUS block, both heads
                        # via column tiling: head A dims on partitions 0:64
                        # (tile T0), head B on 64:128 (tile T1). One bank.
                        av = psA.tile([128, 512], F32, tag="av", name=f"av{g}_{icn}")
                        acc = accp.tile([128, 2, 512], BF16, tag="acc", name=f"ac{g}_{icn}")
                    pts = []
                    # 2-jt macro steps: group same-tile-mode matmuls (4 score
                    # MMs, then projections, then 4 AV MMs) so the PE array
                    # re-tiles 3 times per TWO key-tiles instead of per one.
                    for jm in range(NJT // 2):
                        sts = []
                        for h in range(2):
                            jt = 2 * jm + h
                            jsl = slice(jt * 128, (jt + 1) * 128)
                            # S^T[j, i] for both heads into one 2-bank PSUM
                            # tile; heads run as concurrent 64x128 row tiles.
                            st2 = ps2.tile([128, 2, 512], F32, tag="st2", name=f"st{g}_{icn}_{jt}")
                            nc.tensor.matmul(st2[:, 0, :], kt_t[0:64, jsl], qt_t[0:64, isl])
                            nc.tensor.matmul(st2[:, 1, :], kt_t[64:128, jsl], qt_t[64:128, isl])
                            sts.append(st2)
                        for h in range(2):
                            jt = 2 * jm + h
                            # One exp instruction covers both heads (1024 free).
                            pt2 = ptp.tile([128, 2, 512], BF16, tag="pt", name=f"pt{g}_{icn}_{jt}")
                            if "noact" in ablate:
                                nc.scalar.copy(pt2[:, 0, 0:2], sts[h][:, 0, 0:2])
                            else:
                                nc.scalar.activation(pt2[:, :, :], sts[h][:, :, :], EXP, scale=SCALE)
                            pts.append(pt2)
                        # Interleave next pair's projections while ACT runs
                        # (~2 proj MMs per key-tile; each yield emits two
                        # matmuls), then background V-projection units. More
                        # during the fill block, which has no consume work.
                        for _ in range(2 if prev_blk is not None else 8):
                            next(kq_gen, None)
                        if jm % 2 == 1:
                            bg_pull(1)
                        if prev_blk is not None:
                            consume_step(prev_blk, av, acc, 2 * jm)
                            consume_step(prev_blk, av, acc, 2 * jm + 1)
                        if jm == 1 and pending_mul is not None:
                            pending_mul()
                            pending_mul = None
                    if prev_blk is not None:
                        pending_mul = emit_tail(prev_blk["g"], prev_blk["isl"], av, acc)
                    prev_blk = {"g": g, "isl": isl, "pts": pts}
                for _ in kq_gen:
                    pass
            if "att" not in ablate:
                bg_pull(1 << 30)
                # Drain: consume the final block (no produce side left).
                av = psA.tile([128, 512], F32, tag="av", name="av_drain")
                acc = accp.tile([128, 2, 512], BF16, tag="acc", name="ac_drain")
                for jm in range(NJT // 2):
                    consume_step(prev_blk, av, acc, 2 * jm)
                    consume_step(prev_blk, av, acc, 2 * jm + 1)
                    if jm == 1 and pending_mul is not None:
                        pending_mul()
                        pending_mul = None
                emit_tail(prev_blk["g"], prev_blk["isl"], av, acc)()

            if "out" in ablate:
                ob0 = obp.tile([128, 512], F32, tag="ob", name="ob0")
                nc.vector.tensor_copy(ob0, ot_t[:, 0, 0:512])
                nc.sync.dma_start(out=outT_r[0, :, 0:512], in_=ob0)
                continue
            # Output projection: outT[e, i] = Wo @ O^T + bo. Both query halves
            # share each wo stationary (adjacent matmuls, dedupe).
            wo_t = const.tile([128, KC, C], BF16, tag="xw")
            nc.sync.dma_start(out=wo_t, in_=woT[:, :, :])
            for et in range(C // 128):
                pos = [
                    pp.tile([128, 512], F32, tag="ps", name=f"po{et}_{icn}")
                    for icn in range(TQ // 512)
                ]
                for fc in range(KC):
                    for icn in range(TQ // 512):
                        nc.tensor.matmul(
                            pos[icn],
                            wo_t[:, fc, et * 128 : (et + 1) * 128],
                            ot_t[:, fc, icn * 512 : (icn + 1) * 512],
                            start=(fc == 0),
                            stop=(fc == KC - 1),
                        )
                for icn in range(TQ // 512):
                    ob = obp.tile([128, 512], F32, tag="ob", name=f"o{et}_{icn}")
                    nc.vector.tensor_scalar_add(ob, pos[icn], bo_t[:, et : et + 1])
                    nc.sync.dma_start(
                        out=outT_r[et, :, icn * 512 : (icn + 1) * 512], in_=ob
                    )

    _dedupe_ldweights(nc)
    nc.finalize()
    return nc


_CACHE = {}


def _get_nc():
    if "nc" not in _CACHE:
        _CACHE["nc"] = build_bass()
    return _CACHE["nc"]


def make_in_maps(x, Wq, Wk, Wv, Wo, bo):
    """Host-side sharding: layout prep only (transposes / concatenation)."""
    x = np.asarray(x, dtype=np.float32)
    # Weights pre-tiled into the exact SBUF layouts (contiguous DMAs).
    # wk/wq: [g, p, kc, o] = W[g*128+o, kc*128+p]
    wkT = np.ascontiguousarray(
        np.asarray(Wk, np.float32).reshape(PAIRS, 128, KC, 128).transpose(0, 3, 2, 1)
    ).astype(BF)
    wqT = np.ascontiguousarray(
        np.asarray(Wq, np.float32).reshape(PAIRS, 128, KC, 128).transpose(0, 3, 2, 1)
    ).astype(BF)
    # wv: [p, kc, grp, col] where grp covers 4 heads x 64 dims = 256 cols
    wvT = np.ascontiguousarray(
        np.asarray(Wv, np.float32).T.reshape(KC, 128, 4, 256).transpose(1, 0, 2, 3)
    ).astype(BF)
    # wo: [p, fc, e] = Wo[e, fc*128+p]
    woT = np.ascontiguousarray(
        np.asarray(Wo, np.float32).T.reshape(KC, 128, C).transpose(1, 0, 2)
    ).astype(BF)
    bo = np.ascontiguousarray(np.asarray(bo, np.float32))
    ones_np = np.zeros((128, 128), BF)
    ones_np[:, 0] = 1  # denominator stationary A: head A sum -> partition 0
    ones_np[:, 65] = 1  # denominator stationary B: head B sum -> partition 1
    in_maps = []
    for core in range(8):
        b, qh = core // 2, core % 2
        xb = x[b]
        # My query half first; key/value order is permutation-invariant.
        xrot = np.concatenate([xb[qh * TQ : (qh + 1) * TQ], xb[(1 - qh) * TQ : (2 - qh) * TQ]], axis=0)
        xT_np = np.ascontiguousarray(xrot.T).astype(BF)
        in_maps.append(
            {
                "xT": xT_np,
                "wkT": wkT,
                "wqT": wqT,
                "wvT": wvT,
                "woT": woT,
                "bo": bo,
                "ones_in": ones_np,
            }
        )
    return in_maps


def gather_out(results):
    out = np.empty((B, N, C), dtype=np.float32)
    for core in range(8):
        b, qh = core // 2, core % 2
        out[b, qh * TQ : (qh + 1) * TQ, :] = results[core]["outT"].T
    return out


def kernel(x, Wq, Wk, Wv, Wo, bo):
    from concourse.bass_utils import run_bass_kernel_spmd

    in_maps = make_in_maps(x, Wq, Wk, Wv, Wo, bo)
    res = run_bass_kernel_spmd(_get_nc(), in_maps, core_ids=list(range(8)))
    return gather_out(res.results)


# revision 18
# speedup vs baseline: 1.1002x; 1.0005x over previous
"""Trainium2 Bass kernel for multi-head attention (B=4, N=2048, C=1024, H=16).

Sharding: 8 cores = (batch b, query-half qh). Each core computes attention for
its 1024 query tokens of batch b against all 2048 keys of batch b, all 16
heads, plus the output projection. Host-side work is layout only (transpose /
concat); all FLOPs run on device.

Per-core layout: activations are feature-major ("xT" = [C, tokens]) so every
matmul contracts over the partition axis. Scores are computed transposed
(ST[j keys, i queries]) which makes softmax denominators cheap and makes P@V
need no transpose of P. Softmax skips max-subtraction (|S| <~ 25 for this
distribution, exp is safe in fp32). All matmul operands are bf16.

PE array tiling: the score matmuls contract over D=64, so the two heads of a
pair run as concurrent 64x128 row tiles (T0/T8, auto-derived from the base
partitions). The AV matmuls have 64 stationary columns per head, so the two
heads run as concurrent 128x64 column tiles (T0/T1) accumulating into one
[128,512] PSUM bank - head A dims on partitions 0:64, head B on 64:128,
which is exactly the O^T layout the output projection wants. The softmax
denominator (which previously rode along as a 65th ones-column of V, blocking
column tiling) is instead accumulated per-jt on the DVE (bf16 adds at 2x
rate) and collapsed across partitions by a single ones-stationary matmul per
(pair, query-block).

Matmuls that share a stationary operand are emitted back-to-back and a
post-schedule pass drops the redundant Ldweights (walrus emits one per
matmul unconditionally).
"""

import sys

import ml_dtypes
import numpy as np

BF = ml_dtypes.bfloat16

sys.path.insert(0, "/opt/trn_rl_repo")

import concourse.bass as bass  # noqa: E402
import concourse.bacc as bacc  # noqa: E402
import concourse.mybir as mybir  # noqa: E402
import concourse.tile as tile  # noqa: E402

# Problem constants (hardcoded by contract).
B, N, C = 4, 2048, 1024
H, D = 16, 64
SCALE = D**-0.5  # 0.125
T = N  # key tokens per core
TQ = N // 2  # query tokens per core
KC = C // 128  # contraction chunks of 128
PAIRS = H // 2  # head pairs; pair g = heads (2g, 2g+1)
NJT = T // 128  # 16 key tiles
F32 = mybir.dt.float32
BF16 = mybir.dt.bfloat16
EXP = mybir.ActivationFunctionType.Exp

_KEEP_PE = ("Matmult", "EventSemaphore", "Ldweights")


def _dedupe_ldweights(nc):
    """Drop Ldweights that reload the exact weights already resident.

    Runs after tile scheduling (program order fixed, waits materialized) and
    before finalize. An Ldweights is redundant iff it has the same source AP /
    tile position as the previous kept one, carries no waits or updates of its
    own, and its dependencies are a subset of the kept load's. Only Matmult
    (non-self-loading for bf16) and EventSemaphore may sit in between; any
    other PE instruction conservatively resets the tracked state.
    """
    removed = 0
    for b in nc.main_func.blocks:
        newlist = []
        last = None
        for i in b.instructions:
            if i.opcode == "Ldweights":
                ap = i.ins[0]
                sig = (
                    ap.memsetref,
                    ap.memref,
                    ap.offset,
                    str(ap.ap),
                    str(ap.dtype),
                    str(i.tile_position),
                    str(i.perf_mode),
                    str(getattr(i, "is_transpose", None)),
                )
                deps = {n for n, _ in i.sync_dependencies()}
                deps |= {n for n, _ in i.nosync_dependencies()}
                has_wait = bool(i.sync_info and i.sync_info.on_wait)
                has_upd = bool(i.sync_info and i.sync_info.on_update)
                if (
                    last is not None
                    and sig == last[0]
                    and not has_wait
                    and not has_upd
                    and deps <= last[1]
                ):
                    removed += 1
                    continue
                last = (sig, deps)
            elif i.engine == mybir.EngineType.PE and i.opcode not in _KEEP_PE:
                last = None
            newlist.append(i)
        b.instructions[:] = newlist
    return removed


def build_bass(reps=1, loop_iters=0, ablate=()):
    from contextlib import ExitStack

    nc = bacc.Bacc()
    xT = nc.dram_tensor("xT", [C, T], BF16, kind="ExternalInput")
    wkT = nc.dram_tensor("wkT", [PAIRS, 128, KC, 128], BF16, kind="ExternalInput")
    wqT = nc.dram_tensor("wqT", [PAIRS, 128, KC, 128], BF16, kind="ExternalInput")
    wvT = nc.dram_tensor("wvT", [128, KC, 4, 256], BF16, kind="ExternalInput")
    woT = nc.dram_tensor("woT", [128, KC, C], BF16, kind="ExternalInput")
    bo = nc.dram_tensor("bo", [C], F32, kind="ExternalInput")
    # ones_in[:, 0] = 1 and ones_in[:, 65] = 1, zeros elsewhere: slices
    # [:, 0:64] / [:, 64:128] are the denominator matmul stationaries that
    # place head A's / head B's key-sum on output partitions 0 / 1.
    ones_in = nc.dram_tensor("ones_in", [128, 128], BF16, kind="ExternalInput")
    outT = nc.dram_tensor("outT", [C, TQ], F32, kind="ExternalOutput")

    xT_r = xT.rearrange("(kc p) t -> p kc t", p=128)
    bo_r = bo.rearrange("(a p) -> p a", p=128)
    outT_r = outT.rearrange("(et p) i -> et p i", p=128)

    with tile.TileContext(nc) as tc, ExitStack() as ctx:
        ctx.enter_context(
            nc.allow_low_precision(reason="bf16 matmul path is within error budget")
        )
        const = ctx.enter_context(tc.tile_pool(name="const", bufs=1))
        wpool = ctx.enter_context(tc.tile_pool(name="wpool", bufs=2))
        wvp = ctx.enter_context(tc.tile_pool(name="wvp", bufs=1))
        kqp = ctx.enter_context(tc.tile_pool(name="kqp", bufs=2))
        vpool = ctx.enter_context(tc.tile_pool(name="vpool", bufs=1))
        # pt tiles live for a full block (16 jt) plus the production margin of
        # the next block: the consume side (AV + denominator adds) lags one
        # whole (pair, query-block) behind the produce side (ST + exp) so no
        # consumer ever waits on a just-produced tile.
        ptp = ctx.enter_context(tc.tile_pool(name="ptp", bufs=NJT + 4))
        accp = ctx.enter_context(tc.tile_pool(name="accp", bufs=2))
        smp = ctx.enter_context(tc.tile_pool(name="smp", bufs=2))
        obp = ctx.enter_context(tc.tile_pool(name="obp", bufs=2))
        # PSUM budget (8 banks): st2 2x[128,2,512] = 4, av 1, dn 1, proj 2.
        pp = ctx.enter_context(tc.tile_pool(name="pp", bufs=2, space="PSUM"))
        psA = ctx.enter_context(tc.tile_pool(name="psA", bufs=1, space="PSUM"))
        ps2 = ctx.enter_context(tc.tile_pool(name="ps2", bufs=2, space="PSUM"))
        drp = ctx.enter_context(tc.tile_pool(name="drp", bufs=2, space="DRAM"))

        import contextlib
        loop_ctx = (
            tc.For_i(0, loop_iters, 1) if loop_iters else contextlib.nullcontext()
        )
        with loop_ctx:
          for _rep in range(reps):
            # Constants first (small), then xT per c-chunk so the first
            # projection matmuls start as soon as chunk 0 lands.
            ones = const.tile([128, 128], BF16, tag="ones")
            nc.sync.dma_start(out=ones, in_=ones_in[:, :])
            bo_t = const.tile([128, KC], F32, tag="bo")
            nc.sync.dma_start(out=bo_t, in_=bo_r)
            xt = const.tile([128, KC, T], BF16, tag="xw")
            # O^T, concatenated over heads: rows fc*128+p = feature f, cols = query i.
            ot_t = const.tile([128, PAIRS, TQ], BF16, tag="ot")

            def kq_alloc(g):
                """Allocate tiles and start weight DMAs for pair g's k/q projections."""
                t = {}
                t["wk"] = wpool.tile([128, KC, 128], BF16, tag="wk", name=f"wk{g}")
                nc.sync.dma_start(out=t["wk"], in_=wkT[g])
                t["wq"] = wpool.tile([128, KC, 128], BF16, tag="wq", name=f"wq{g}")
                nc.sync.dma_start(out=t["wq"], in_=wqT[g])
                t["kt"] = kqp.tile([128, T], BF16, tag="kT", name=f"kt{g}")
                t["qt"] = kqp.tile([128, TQ], BF16, tag="qT", name=f"qt{g}")
                return t

            def v_emit_front(wv_t, v_t, p):
                """V projection for head-group p in the prologue: PSUM from
                the pp pool, evacuation on the (idle) scalar engine."""
                if "proj" in ablate:
                    nc.vector.tensor_copy(v_t[:, 0, 0:4, :], xt[:, 0, 0:256])
                    return
                for tt2 in range(NJT // 2):
                    pvs = [
                        pp.tile([128, 256], F32, tag="ps", name=f"pv{tt2}_{h}")
                        for h in range(2)
                    ]
                    for kc in range(KC):
                        for h in range(2):
                            tt = 2 * tt2 + h
                            nc.tensor.matmul(
                                pvs[h],
                                xt[:, kc, tt * 128 : (tt + 1) * 128],
                                wv_t[:, kc, p, :],
                                start=(kc == 0),
                                stop=(kc == KC - 1),
                            )
                    for h in range(2):
                        tt = 2 * tt2 + h
                        nc.scalar.copy(
                            v_t[:, tt, 4 * (p % 2) : 4 * (p % 2) + 4, :], pvs[h]
                        )

            def v_emit_bg(wv_t, v_t, p, wide=False):
                """V projection interleaved into attention slack: single-PSUM
                units via the dn-tag bank, evacuation on the DVE (the scalar
                engine is saturated by exp there). With wide=True one unit
                covers head-groups p and p+1 via a 512-wide moving operand
                (same stationary, half the Ldweights)."""
                if "proj" in ablate:
                    return
                ncols = 512 if wide else 256
                for tt in range(NJT):
                    # One pull = one whole unit (8 accumulating MMs + copy):
                    # a unit must never straddle a block boundary, where
                    # emit_tail allocates the same dn-tag PSUM slab - a split
                    # would deadlock the in-order PE queue on the ring.
                    pv = psA.tile([128, ncols], F32, tag="dn", name=f"pvb{p}_{tt}")
                    for kc in range(KC):
                        mov = (wv_t[:, kc, p : p + 2, :].rearrange("p a b -> p (a b)")
                               if wide else wv_t[:, kc, p, :])
                        nc.tensor.matmul(
                            pv,
                            xt[:, kc, tt * 128 : (tt + 1) * 128],
                            mov,
                            start=(kc == 0),
                            stop=(kc == KC - 1),
                        )
                    if wide:
                        nc.vector.tensor_copy(v_t[:, tt, :, :], pv)
                    else:
                        nc.vector.tensor_copy(
                            v_t[:, tt, 4 * (p % 2) : 4 * (p % 2) + 4, :], pv
                        )
                    yield

            def kq_emit(g, t):
                if "proj" in ablate:
                    nc.vector.tensor_copy(t["kt"][:, 0:512], xt[:, 0, 0:512])
                    nc.vector.tensor_copy(t["qt"][:, 0:512], xt[:, 1, 0:512])
                    yield
                    return
                kt_t = t["kt"]
                # Token-pair groups: one wk stationary serves 2 adjacent
                # matmuls (dedupe drops every other Ldweights).
                for tcn2 in range(T // 1024):
                    pks = [
                        pp.tile([128, 512], F32, tag="ps", name=f"pk{g}_{tcn2}_{h}")
                        for h in range(2)
                    ]
                    for kc in range(KC):
                        for h in range(2):
                            tcn = tcn2 * 2 + h
                            nc.tensor.matmul(
                                pks[h],
                                t["wk"][:, kc, :],
                                xt[:, kc, tcn * 512 : (tcn + 1) * 512],
                                start=(kc == 0),
                                stop=(kc == KC - 1),
                            )
                        yield
                    for h in range(2):
                        tcn = tcn2 * 2 + h
                        nc.vector.tensor_copy(
                            kt_t[:, tcn * 512 : (tcn + 1) * 512], pks[h]
                        )
                        yield
                qt_t = t["qt"]
                pqs = [
                    pp.tile([128, 512], F32, tag="ps", name=f"pq{g}_{icn}")
                    for icn in range(TQ // 512)
                ]
                for kc in range(KC):
                    for icn in range(TQ // 512):
                        nc.tensor.matmul(
                            pqs[icn],
                            t["wq"][:, kc, :],
                            xt[:, kc, icn * 512 : (icn + 1) * 512],
                            start=(kc == 0),
                            stop=(kc == KC - 1),
                        )
                    yield
                for icn in range(TQ // 512):
                    nc.vector.tensor_copy(
                        qt_t[:, icn * 512 : (icn + 1) * 512], pqs[icn]
                    )
                    yield

            # Prologue: only head-group 0 of the V projection (pairs 0-1)
            # plus pair 0's K/Q run before attention starts; V groups 1-3 are
            # interleaved into attention slack behind the K/Q projections.
            # Two separate V tiles so early pairs' AV matmuls never wait on
            # late V writes.
            tiles = [None] * PAIRS
            tiles[0] = kq_alloc(0)
            wv_t = wvp.tile([128, KC, 4, 256], BF16, tag="wv")
            nc.sync.dma_start(out=wv_t, in_=wvT[:, :, :, :])
            # V per head, 64-wide (no ones column): [p, jt, head-of-half, d].
            v01 = vpool.tile([128, NJT, H // 2, 64], BF16, tag="v01")
            v23 = vpool.tile([128, NJT, H // 2, 64], BF16, tag="v23")
            for kc in range(KC):
                nc.sync.dma_start(out=xt[:, kc, :], in_=xT_r[:, kc, :])
            v_emit_front(wv_t, v01, 0)
            if "proj" in ablate:
                nc.vector.tensor_copy(v23[:, 0, 0:4, :], xt[:, 0, 0:256])
            for _ in kq_emit(0, tiles[0]):
                pass
            from collections import deque
            bg = deque()
            bg.append(v_emit_bg(wv_t, v01, 1))
            bg.append(v_emit_bg(wv_t, v23, 2, wide=True))

            def bg_pull(n):
                for _ in range(n):
                    while bg:
                        try:
                            next(bg[0])
                            break
                        except StopIteration:
                            bg.popleft()
                    else:
                        return

            def emit_tail(g, isl, av, acc):
                """Denominator + evacuation for one finished (pair, query-block).

                Emitted at the end of the block AFTER the one it belongs to
                (the consume side lags a full block), so acc and av are long
                done. Returns a closure for the final normalize mul, to be
                invoked a couple of steps later still - by then the
                reciprocal's DRAM-broadcast bounce has landed, so the DVE
                never blocks on it.
                """
                if "noepi" in ablate:
                    av_sb = smp.tile([128, 512], BF16, tag="avs", name=f"as{g}_{isl.start}")
                    nc.vector.tensor_copy(av_sb, av)
                    nc.vector.tensor_copy(ot_t[:, g, isl], av_sb)
                    return lambda: None
                # Collapse acc across the 128 key partitions with
                # ones-stationary matmuls; head A lands on output partition 0,
                # head B on partition 1 (128x64 mode, same as AV).
                dn = psA.tile([64, 512], F32, tag="dn", name=f"dn{g}_{isl.start}")
                nc.tensor.matmul(dn, ones[:, 0:64], acc[:, 0, :],
                                 start=True, stop=False)
                nc.tensor.matmul(dn, ones[:, 64:128], acc[:, 1, :],
                                 start=False, stop=True)
                # Copy accumulator to SBUF right away to free the av bank.
                av_sb = smp.tile([128, 512], BF16, tag="avs", name=f"as{g}_{isl.start}")
                nc.vector.tensor_copy(av_sb, av)
                rec = smp.tile([2, 512], BF16, tag="rec", name=f"rc{g}_{isl.start}")
                nc.vector.reciprocal(rec, dn[0:2, :])
                # Bounce through DRAM and broadcast-load across partitions
                # (stride-0 DRAM AP): rows 0:64 get 1/dA, 64:128 get 1/dB.
                rec_d = drp.tile([2, 512], BF16, tag="rd", name=f"rd{g}_{isl.start}")
                nc.sync.dma_start(out=rec_d, in_=rec)
                bc = smp.tile([128, 512], BF16, tag="bc", name=f"bc{g}_{isl.start}")
                nc.sync.dma_start(
                    out=bc[0:64, :],
                    in_=bass.AP(tensor=rec_d[:, :].tensor, offset=rec_d[0:1, :].offset,
                                ap=[[0, 64], [1, 512]]),
                )
                nc.sync.dma_start(
                    out=bc[64:128, :],
                    in_=bass.AP(tensor=rec_d[:, :].tensor, offset=rec_d[1:2, :].offset,
                                ap=[[0, 64], [1, 512]]),
                )

                def mul():
                    # Normalize both heads straight into O^T layout.
                    nc.vector.tensor_mul(ot_t[:, g, isl], av_sb, bc)

                return mul

            def consume_step(blk, av, acc, jt):
                """One lagged consume step: AV matmuls + denominator add for
                key-tile jt of the previous (pair, query-block)."""
                pg = blk["g"]
                pv = blk["pts"][jt]
                v_t = v01 if pg < 4 else v23
                vh = (2 * pg) % 8
                if "noav" not in ablate:
                    nc.tensor.matmul(
                        av[0:64, :], v_t[:, jt, vh, :], pv[:, 0, :],
                        start=(jt == 0), stop=(jt == NJT - 1),
                        skip_group_check=True,
                    )
                    nc.tensor.matmul(
                        av[64:128, :], v_t[:, jt, vh + 1, :], pv[:, 1, :],
                        start=(jt == 0), stop=(jt == NJT - 1),
                        skip_group_check=True,
                    )
                elif jt == 0:
                    nc.vector.tensor_copy(av[0:2, 0:2], pv[0:2, 0, 0:2])
                if "nodve" in ablate:
                    if jt == 0:
                        nc.vector.tensor_copy(acc[:, 0, 0:2], pv[:, 0, 0:2])
                elif jt == 0:
                    nc.vector.tensor_copy(acc, pv)
                else:
                    nc.vector.tensor_add(acc, acc, pv)

            # The attention pipeline: the produce side of block k (scores +
            # exp) runs while the consume side (AV accumulation, denominator
            # adds, epilogue) works on block k-1. Every cross-engine
            # dependency is therefore ~16 steps stale and no engine queue
            # ever blocks another.
            prev_blk = None
            pending_mul = None
            pending_tail = None
            for g in range(PAIRS):
                t = tiles[g]
                kt_t, qt_t = t["kt"], t["qt"]
                if g + 1 < PAIRS:
                    tiles[g + 1] = kq_alloc(g + 1)
                    kq_gen = kq_emit(g + 1, tiles[g + 1])
                else:
                    kq_gen = iter(())

                if "att" in ablate:
                    for _ in kq_gen:
                        pass
                    nc.vector.tensor_copy(ot_t[0:64, g, 0:512], kt_t[0:64, 0:512])
                    continue
                for icn in range(TQ // 512):
                    isl = slice(icn * 512, (icn + 1) * 512)
                    if prev_blk is not None:
                        # O^T accumulator for the PREVIOUS block, both heads
                        # via column tiling: head A dims on partitions 0:64
                        # (tile T0), head B on 64:128 (tile T1). One bank.
                        av = psA.tile([128, 512], F32, tag="av", name=f"av{g}_{icn}")
                        acc = accp.tile([128, 2, 512], BF16, tag="acc", name=f"ac{g}_{icn}")
                    pts = []
                    # 2-jt macro steps: group same-tile-mode matmuls (4 score
                    # MMs, then projections, then 4 AV MMs) so the PE array
                    # re-tiles 3 times per TWO key-tiles instead of per one.
                    for jm in range(NJT // 2):
                        sts = []
                        for h in range(2):
                            jt = 2 * jm + h
                            jsl = slice(jt * 128, (jt + 1) * 128)
                            # S^T[j, i] for both heads into one 2-bank PSUM
                            # tile; heads run as concurrent 64x128 row tiles.
                            st2 = ps2.tile([128, 2, 512], F32, tag="st2", name=f"st{g}_{icn}_{jt}")
                            nc.tensor.matmul(st2[:, 0, :], kt_t[0:64, jsl], qt_t[0:64, isl])
                            nc.tensor.matmul(st2[:, 1, :], kt_t[64:128, jsl], qt_t[64:128, isl])
                            sts.append(st2)
                        for h in range(2):
                            jt = 2 * jm + h
                            # One exp instruction covers both heads (1024 free).
                            pt2 = ptp.tile([128, 2, 512], BF16, tag="pt", name=f"pt{g}_{icn}_{jt}")
                            if "noact" in ablate:
                                nc.scalar.copy(pt2[:, 0, 0:2], sts[h][:, 0, 0:2])
                            else:
                                nc.scalar.activation(pt2[:, :, :], sts[h][:, :, :], EXP, scale=SCALE)
                            pts.append(pt2)
                        # Interleave next pair's projections while ACT runs
                        # (~2 proj MMs per key-tile; each yield emits two
                        # matmuls), then background V-projection units. More
                        # during the fill block, which has no consume work.
                        for _ in range(2 if prev_blk is not None else 8):
                            next(kq_gen, None)
                        if jm % 2 == 1:
                            bg_pull(1)
                        if prev_blk is not None:
                            consume_step(prev_blk, av, acc, 2 * jm)
                            consume_step(prev_blk, av, acc, 2 * jm + 1)
                        if jm == 1 and pending_mul is not None:
                            pending_mul()
                            pending_mul = None
                    if prev_blk is not None:
                        pending_mul = emit_tail(prev_blk["g"], prev_blk["isl"], av, acc)
                    prev_blk = {"g": g, "isl": isl, "pts": pts}
                for _ in kq_gen:
                    pass

            # Output projection weights reuse xt's SBUF slab (same tag; xt is
            # dead once the interleaved projections finish).
            wo_t = const.tile([128, KC, C], BF16, tag="xw")
            nc.sync.dma_start(out=wo_t, in_=woT[:, :, :])

            def o_emit(icn):
                """Output projection for one query half: per et-tile, 8
                accumulating matmuls over the feature chunks + bias + DMA."""
                for et in range(C // 128):
                    po = pp.tile([128, 512], F32, tag="ps", name=f"po{et}_{icn}")
                    for fc in range(KC):
                        nc.tensor.matmul(
                            po,
                            wo_t[:, fc, et * 128 : (et + 1) * 128],
                            ot_t[:, fc, icn * 512 : (icn + 1) * 512],
                            start=(fc == 0),
                            stop=(fc == KC - 1),
                        )
                        if fc == KC // 2:
                            yield
                    ob = obp.tile([128, 512], F32, tag="ob", name=f"o{et}_{icn}")
                    nc.vector.tensor_scalar_add(ob, po, bo_t[:, et : et + 1])
                    nc.sync.dma_start(
                        out=outT_r[et, :, icn * 512 : (icn + 1) * 512], in_=ob
                    )
                    yield

            o_gen0 = o_emit(0)
            o_gen1 = o_emit(1)
            if "att" not in ablate:
                bg_pull(1 << 30)
                # Drain: consume the final block (no produce side left).
                av = psA.tile([128, 512], F32, tag="av", name="av_drain")
                acc = accp.tile([128, 2, 512], BF16, tag="acc", name="ac_drain")
                for jm in range(NJT // 2):
                    consume_step(prev_blk, av, acc, 2 * jm)
                    consume_step(prev_blk, av, acc, 2 * jm + 1)
                    if jm == 1 and pending_mul is not None:
                        pending_mul()
                        pending_mul = None
                    if jm >= 2:
                        next(o_gen0, None)
                        next(o_gen0, None)
                emit_tail(prev_blk["g"], prev_blk["isl"], av, acc)()
                for _ in o_gen0:
                    pass

            if "out" in ablate:
                ob0 = obp.tile([128, 512], F32, tag="ob", name="ob0")
                nc.vector.tensor_copy(ob0, ot_t[:, 0, 0:512])
                nc.sync.dma_start(out=outT_r[0, :, 0:512], in_=ob0)
                continue
            # Output projection: outT[e, i] = Wo @ O^T + bo. The icn=0 half
            # interleaves into the drain block (its ot inputs complete at
            # drain step 1); the icn=1 half follows the final tail.
            for _ in o_gen1:
                pass

    _dedupe_ldweights(nc)
    nc.finalize()
    return nc


_CACHE = {}


def _get_nc():
    if "nc" not in _CACHE:
        _CACHE["nc"] = build_bass()
    return _CACHE["nc"]


def make_in_maps(x, Wq, Wk, Wv, Wo, bo):
    """Host-side sharding: layout prep only (transposes / concatenation)."""
    x = np.asarray(x, dtype=np.float32)
    # Weights pre-tiled into the exact SBUF layouts (contiguous DMAs).
    # wk/wq: [g, p, kc, o] = W[g*128+o, kc*128+p]
    wkT = np.ascontiguousarray(
        np.asarray(Wk, np.float32).reshape(PAIRS, 128, KC, 128).transpose(0, 3, 2, 1)
    ).astype(BF)
    wqT = np.ascontiguousarray(
        np.asarray(Wq, np.float32).reshape(PAIRS, 128, KC, 128).transpose(0, 3, 2, 1)
    ).astype(BF)
    # wv: [p, kc, grp, col] where grp covers 4 heads x 64 dims = 256 cols
    wvT = np.ascontiguousarray(
        np.asarray(Wv, np.float32).T.reshape(KC, 128, 4, 256).transpose(1, 0, 2, 3)
    ).astype(BF)
    # wo: [p, fc, e] = Wo[e, fc*128+p]
    woT = np.ascontiguousarray(
        np.asarray(Wo, np.float32).T.reshape(KC, 128, C).transpose(1, 0, 2)
    ).astype(BF)
    bo = np.ascontiguousarray(np.asarray(bo, np.float32))
    ones_np = np.zeros((128, 128), BF)
    ones_np[:, 0] = 1  # denominator stationary A: head A sum -> partition 0
    ones_np[:, 65] = 1  # denominator stationary B: head B sum -> partition 1
    in_maps = []
    for core in range(8):
        b, qh = core // 2, core % 2
        xb = x[b]
        # My query half first; key/value order is permutation-invariant.
        xrot = np.concatenate([xb[qh * TQ : (qh + 1) * TQ], xb[(1 - qh) * TQ : (2 - qh) * TQ]], axis=0)
        xT_np = np.ascontiguousarray(xrot.T).astype(BF)
        in_maps.append(
            {
                "xT": xT_np,
                "wkT": wkT,
                "wqT": wqT,
                "wvT": wvT,
                "woT": woT,
                "bo": bo,
                "ones_in": ones_np,
            }
        )
    return in_maps


def gather_out(results):
    out = np.empty((B, N, C), dtype=np.float32)
    for core in range(8):
        b, qh = core // 2, core % 2
        out[b, qh * TQ : (qh + 1) * TQ, :] = results[core]["outT"].T
    return out


def kernel(x, Wq, Wk, Wv, Wo, bo):
    from concourse.bass_utils import run_bass_kernel_spmd

    in_maps = make_in_maps(x, Wq, Wk, Wv, Wo, bo)
    res = run_bass_kernel_spmd(_get_nc(), in_maps, core_ids=list(range(8)))
    return gather_out(res.results)
